# revision 39
# baseline (speedup 1.0000x reference)
"""Trainium2 Bass kernel for nn_MultiHeadAttention (B=4,T=1024,C=1024,H=16).

Sharding: 8 cores = 4 batches x 2 query-halves. Each core computes, for its
batch b and its 512 query rows:
  V projection (natural layout, mask folded in, + 64 replicated mask cols per
  head so the O matmul emits the softmax denominator pre-replicated across 64
  psum partitions), then per head-pair: Q^T/K^T projection chunks,
  S^T = K^T.T @ Q^T (row-packed head pairs, D=64 contraction), E^T =
  exp(0.125*S^T) on ACT (unmasked), O^T+den via one [V_h*m | m*64] matmul,
  normalize with reciprocal_approx_fast at partition base 0 (no DRAM bounce);
  finally Y = O^T.T @ Wo + (bv@Wo+bo) (bias via K=1 matmul) and LayerNorm with
  stats on PSUM, affine on ACT, bf16 output.
Host gathers the 8 [512,1024] bf16 outputs into [4,1024,1024] f32.
"""

import os
import sys

import numpy as np

for _p in ("/opt/trn_rl_repo", "/root/.axon_site/_ro/trn_rl_repo"):
    if os.path.isdir(_p) and _p not in sys.path:
        sys.path.append(_p)

import ml_dtypes  # noqa: E402
import concourse.bass as bass  # noqa: E402
import concourse.mybir as mybir  # noqa: E402
import concourse.tile as tile  # noqa: E402
from concourse import bacc  # noqa: E402
from concourse.bass_utils import run_bass_kernel_spmd  # noqa: E402

BF16 = mybir.dt.bfloat16
F32 = mybir.dt.float32
NPBF16 = ml_dtypes.bfloat16

B, T, C, H = 4, 1024, 1024, 16
D = C // H          # 64
P = 128             # partitions
NC = C // P         # 8 chunks of C
NT = T // P         # 8 chunks of T
TQ = T // 2         # 512 query rows per core
NQ = TQ // P        # 4 query chunks
NPAIR = H // 2      # 8 head pairs
EPS = 1e-5

_CACHE = {}
LAST_RESULTS = None


def _ensure_ntff_hook():
    """Register the axon NTFF profiling hook if the image's antenv lacks it."""
    try:
        import antenv.axon_hooks  # noqa: F401
        return
    except ImportError:
        pass
    try:
        import types

        import antenv
        from trn_agent_boot.trn_boot import _ntff_profile_via_ctypes

        mod = types.ModuleType("antenv.axon_hooks")
        state = {"hook": None}
        mod.set_axon_ntff_profile_hook = lambda h: state.__setitem__("hook", h)
        mod.get_axon_ntff_profile_hook = lambda: state["hook"]
        sys.modules["antenv.axon_hooks"] = mod
        antenv.axon_hooks = mod
        hook = _ntff_profile_via_ctypes("/opt/axon/libaxon_pjrt.so")
        if hook is not None:
            mod.set_axon_ntff_profile_hook(hook)
    except Exception:
        pass


def _emit(nc, tc, dr, NK):
    """Emit the per-core Tile program."""
    from contextlib import ExitStack

    with ExitStack() as ctx:
        consts = ctx.enter_context(tc.tile_pool(name="consts", bufs=1))

        KL = NK * P
        # VA[:, jc, h, :] = [V_h*m | m x64] for even h (O rows 0:64, den64
        # rows 64:128), [m x64 | V_h*m] for odd h (den64 low, O high). The
        # mask block gives the softmax denominator replicated across 64 psum
        # partitions for free in the O matmul.
        VA = consts.tile([P, NK, H, P], BF16)
        OT = consts.tile([P, NC, TQ], BF16)        # O^T (unnormed even/odd)
        Wo_sb = consts.tile([P, NC, C], BF16)
        lng_rep = consts.tile([P, C], BF16)
        lnb_rep = consts.tile([P, C], BF16)
        boe_sb = consts.tile([1, C], BF16)         # bv@Wo + bo
        ones1 = consts.tile([1, P], BF16)
        vecs = consts.tile([P, NC, 3], F32)        # bq | bk | maskf
        maskv = consts.tile([P, NK], BF16)
        eps_t = consts.tile([P, 1], F32)

        nc.vector.memset(eps_t, EPS)
        nc.vector.memset(ones1, 1.0)
        nc.gpsimd.dma_start(out=vecs[:], in_=dr["vecs"].ap()[:])
        nc.gpsimd.dma_start(out=maskv[:], in_=dr["maskv"].ap()[:])
        nc.gpsimd.dma_start(out=boe_sb[:], in_=dr["boe"].ap()[:])

        with (
            tc.tile_pool(name="pa", bufs=1) as pa,
            tc.tile_pool(name="pb", bufs=2) as pb,
            tc.tile_pool(name="psP", bufs=2, space="PSUM") as psP,
        ):
            xT = pa.tile([P, NC, KL], BF16)
            xTq = pa.tile([P, NC, TQ], BF16)
            Wq_sb = pa.tile([P, NC, C], BF16)
            Wk_sb = pa.tile([P, NC, C], BF16)
            Wv_sb = pa.tile([P, NC, C], BF16)
            # Input loads striped across the three DMA queues (sync/scalar/
            # gpsimd each stripe over all 16 DMA engines; aggregate ~350GB/s),
            # phased by first-use: xT+Wv lo | Wv hi | xTq+Wq | Wk | Wo.
            ENG = [nc.sync, nc.scalar, nc.gpsimd]

            def _ldsplit(dst, src):
                n = dst.shape[1]
                bnd = [0, (n + 2) // 3, n - (n + 2) // 3, n]
                bnd[2] += bnd[1]
                for e in range(3):
                    lo, hi = bnd[e], bnd[e + 1]
                    if hi > lo:
                        ENG[e].dma_start(
                            out=dst[:, lo:hi], in_=src[:, lo:hi]
                        )

            # V-proj inputs interleaved per kc chunk so mm(kc) can start as
            # soon as its own xT row + Wv column block land (~10.5us).
            for kc in range(NC):
                ENG[kc % 3].dma_start(
                    out=xT[:, kc, :], in_=dr["xT"].ap()[:, kc, :]
                )
                ENG[kc % 3].dma_start(
                    out=Wv_sb[:, kc, 0:TQ], in_=dr["Wv"].ap()[:, kc, 0:TQ]
                )
            _ldsplit(Wv_sb[:, :, TQ:], dr["Wv"].ap()[:, :, TQ:])
            _ldsplit(Wk_sb[:], dr["Wk"].ap()[:])
            _ldsplit(xTq[:], dr["xTq"].ap()[:])
            _ldsplit(Wq_sb[:], dr["Wq"].ap()[:])
            _ldsplit(Wo_sb[:], dr["Wo"].ap()[:])
            for name, rep in (("lng", lng_rep), ("lnb", lnb_rep)):
                a = dr[name].ap()
                bcast = bass.AP(
                    tensor=a.tensor, offset=a.offset, ap=[[0, P], [1, C]]
                )
                nc.gpsimd.dma_start(out=rep[:], in_=bcast)

            # ---- output projection + LayerNorm body (called per qc) ----
            def _qc_head(qc, psn):
                # mc 0..6 accumulation for both halves (no reads of the last
                # pair's OT chunk) — bridge work for the tensor queue.
                qs = slice(qc * P, (qc + 1) * P)
                for nn in range(2):
                    for mc in range(NC - 1):
                        nc.tensor.matmul(
                            psn[nn],
                            OT[:, mc, qs],
                            Wo_sb[:, mc, nn * TQ : (nn + 1) * TQ],
                            start=(mc == 0),
                            stop=False,
                        )

            def _qc_tail(qc, psn):
                qs = slice(qc * P, (qc + 1) * P)
                for nn in range(2):
                    nc.tensor.matmul(
                        psn[nn],
                        OT[:, NC - 1, qs],
                        Wo_sb[:, NC - 1, nn * TQ : (nn + 1) * TQ],
                        start=False, stop=False,
                    )
                    nc.tensor.matmul(
                        psn[nn],
                        ones1[0:1, :],
                        boe_sb[0:1, nn * TQ : (nn + 1) * TQ],
                        start=False, stop=True,
                    )
                stats = pb.tile(
                    [P, 2, nc.vector.BN_STATS_DIM], F32, tag="stats"
                )
                mv = pb.tile([P, nc.vector.BN_AGGR_DIM], F32, tag="mv")
                nc.vector.bn_stats(out=stats[:, 0, :], in_=psn[0][:])
                nc.vector.bn_stats(out=stats[:, 1, :], in_=psn[1][:])
                nc.vector.bn_aggr(out=mv[:], in_=stats[:])
                rstd = pb.tile([P, 1], F32, tag="rstd")
                nmr = pb.tile([P, 1], F32, tag="nmr")
                nc.scalar.activation(
                    out=rstd[:], in_=mv[:, 1:2],
                    func=mybir.ActivationFunctionType.Sqrt,
                    bias=eps_t[:], scale=1.0,
                )
                nc.vector.reciprocal(out=rstd[:], in_=rstd[:])
                nc.vector.tensor_scalar(
                    nmr[:], mv[:, 0:1], rstd[:], -1.0,
                    mybir.AluOpType.mult, mybir.AluOpType.mult,
                )
                Yf = pb.tile([P, C], BF16, tag="yf")
                for nn in range(2):
                    sl = slice(nn * TQ, (nn + 1) * TQ)
                    nc.scalar.activation(
                        out=Yf[:, sl], in_=psn[nn][:],
                        func=mybir.ActivationFunctionType.Identity,
                        bias=nmr[:], scale=rstd[:],
                    )
                    nc.vector.tensor_tensor(
                        Yf[:, sl], Yf[:, sl], lng_rep[:, sl],
                        mybir.AluOpType.mult,
                    )
                    nc.vector.tensor_tensor(
                        Yf[:, sl], Yf[:, sl], lnb_rep[:, sl],
                        mybir.AluOpType.add,
                    )
                    (nc.sync if nn == 0 else nc.gpsimd).dma_start(
                        out=dr["y"].ap()[qs, sl], in_=Yf[:, sl]
                    )

            with (
                tc.tile_pool(name="psS", bufs=2, space="PSUM") as psS,
                tc.tile_pool(name="psO", bufs=2, space="PSUM") as psO,
            ):
                # ---- V projection: [keys, C] natural, masked, + mask block --
                for nn in range(2):
                    for tcn in range(NK):
                        ps = psP.tile([P, TQ], F32, tag="psp")
                        for kc in range(NC):
                            nc.tensor.matmul(
                                ps[:],
                                xT[:, kc, tcn * P : (tcn + 1) * P],
                                Wv_sb[:, kc, nn * TQ : (nn + 1) * TQ],
                                start=(kc == 0),
                                stop=(kc == NC - 1),
                            )
                        # V blocks land at +0 (even heads) / +64 (odd heads)
                        a = VA[:, tcn, :, :]
                        vdst = bass.AP(
                            tensor=a.tensor, offset=a.offset + nn * 8 * P,
                            ap=[a.ap[0], [2 * P, 4], [P + D, 2], [1, D]],
                        )
                        nc.vector.tensor_scalar_mul(
                            vdst,
                            ps[:].rearrange("p (a b d) -> p a b d", a=4, b=2),
                            vecs[:, tcn, 2:3],
                        )
                        if nn == 0:
                            # mask blocks: +64 for even heads, +0 for odd
                            mdst = bass.AP(
                                tensor=a.tensor, offset=a.offset + D,
                                ap=[a.ap[0], [2 * P, 8], [D, 2], [1, D]],
                            )
                            nc.vector.tensor_copy(
                                out=mdst,
                                in_=maskv[:, tcn, None].to_broadcast(
                                    (P, 8, 2, D)
                                ),
                            )

                # ---- per head-pair: QT/KT proj, S^T, exp, O^T, normalize,
                # software-pipelined so K/Q(c+1) matmuls cover exp(c) on ACT.
                def emit_kq(c):
                    QTc = pb.tile([P, TQ], BF16, tag="qtc")
                    KTc = pb.tile([P, KL], BF16, tag="ktc")
                    for ko in range(0, KL, TQ):
                        w = min(TQ, KL - ko)
                        ps = psP.tile([P, TQ], F32, tag="psp")
                        for kc in range(NC):
                            nc.tensor.matmul(
                                ps[:, :w],
                                Wk_sb[:, kc, c * P : (c + 1) * P],
                                xT[:, kc, ko : ko + w],
                                start=(kc == 0),
                                stop=(kc == NC - 1),
                            )
                        nc.vector.tensor_scalar_add(
                            KTc[:, ko : ko + w], ps[:, :w], vecs[:, c, 1:2]
                        )
                    ps = psP.tile([P, TQ], F32, tag="psp")
                    for kc in range(NC):
                        nc.tensor.matmul(
                            ps[:],
                            Wq_sb[:, kc, c * P : (c + 1) * P],
                            xTq[:, kc, :],
                            start=(kc == 0),
                            stop=(kc == NC - 1),
                        )
                    nc.vector.tensor_scalar_add(QTc[:], ps[:], vecs[:, c, 0:1])
                    return QTc, KTc

                def emit_s(c, QTc, KTc):
                    et0 = pb.tile([P, NK, TQ], BF16, tag="et0", bufs=2)
                    et1 = pb.tile([P, NK, TQ], BF16, tag="et1", bufs=2)
                    for jj in range((NK + 1) // 2):
                        nu = min(2, NK - 2 * jj)
                        s0 = psS.tile([P, 2 * TQ], F32, tag="s0", bufs=1)
                        s1 = psS.tile([P, 2 * TQ], F32, tag="s1", bufs=1)
                        for u in range(nu):
                            jc = 2 * jj + u
                            js = slice(jc * P, (jc + 1) * P)
                            nc.tensor.matmul(
                                s0[:, u * TQ : (u + 1) * TQ],
                                KTc[0:D, js],
                                QTc[0:D, :],
                                start=True, stop=True,
                                tile_position=(0, 0),
                            )
                            nc.tensor.matmul(
                                s1[:, u * TQ : (u + 1) * TQ],
                                KTc[D:P, js],
                                QTc[D:P, :],
                                start=True, stop=True,
                                tile_position=(D, 0),
                            )
                        nc.scalar.activation(
                            out=et0[:, 2 * jj : 2 * jj + nu, :],
                            in_=s0[:, : nu * TQ],
                            func=mybir.ActivationFunctionType.Exp, scale=0.125,
                        )
                        nc.scalar.activation(
                            out=et1[:, 2 * jj : 2 * jj + nu, :],
                            in_=s1[:, : nu * TQ],
                            func=mybir.ActivationFunctionType.Exp, scale=0.125,
                        )
                    return et0, et1

                def emit_o_den(c, et0, et1):
                    h0, h1 = 2 * c, 2 * c + 1
                    # O^T + den64: even head [O | den64], odd [den64 | O].
                    # The even head's den chain (ACT copy -> DMA shift ->
                    # recip) is emitted between the two O matmul groups so it
                    # overlaps the odd head's matmuls.
                    po0 = psO.tile([P, TQ], F32, tag="po0", bufs=1)
                    po1 = psO.tile([P, TQ], F32, tag="po1", bufs=1)
                    dt = pb.tile([P, 2, TQ], F32, tag="dt")
                    rr = pb.tile([P, 2, TQ], F32, tag="rr")
                    for jc in range(NK):
                        nc.tensor.matmul(
                            po0[:], VA[:, jc, h0, :], et0[:, jc, :],
                            start=(jc == 0), stop=(jc == NK - 1),
                        )
                    nc.scalar.activation(
                        out=dt[D:P, 0, :], in_=po0[D:P, :],
                        func=mybir.ActivationFunctionType.Identity,
                    )
                    nc.sync.dma_start(out=dt[0:D, 0, :], in_=dt[D:P, 0, :])
                    nc.vector.reciprocal_approx_fast(
                        out=rr[0:D, 0, :], in_=dt[0:D, 0, :]
                    )
                    for jc in range(NK):
                        nc.tensor.matmul(
                            po1[:], VA[:, jc, h1, :], et1[:, jc, :],
                            start=(jc == 0), stop=(jc == NK - 1),
                        )
                    nc.vector.reciprocal_approx_fast(
                        out=rr[0:D, 1, :], in_=po1[0:D, :]
                    )
                    nc.sync.dma_start(out=rr[D:P, 1, :], in_=rr[0:D, 1, :])
                    nc.vector.tensor_tensor(
                        OT[0:D, c, :], po0[0:D, :], rr[0:D, 0, :],
                        mybir.AluOpType.mult,
                    )
                    nc.vector.tensor_tensor(
                        OT[D:P, c, :], po1[D:P, :], rr[D:P, 1, :],
                        mybir.AluOpType.mult,
                    )

                ets = emit_s(0, *emit_kq(0))
                for c in range(NPAIR):
                    nxt = emit_kq(c + 1) if c + 1 < NPAIR else None
                    emit_o_den(c, *ets)
                    if nxt is not None:
                        ets = emit_s(c + 1, *nxt)

                # qc 0/1 emitted inside the psS/psO scope from psP so their
                # matmuls bridge the pool-close barrier (which waits on the
                # last pair's den chain); the rest pipeline deeply from psL.
                # Bridge the pool-close barrier: four mc0..6 chains emitted
                # before any mc7 tail (the in-order tensor queue would
                # otherwise block on the last pair's OT chunk). qc1's psums
                # reuse the psS slots, free right after pair 7's exp.
                a0 = psP.tile([P, TQ], F32, tag="psp")
                a1 = psP.tile([P, TQ], F32, tag="psp")
                b0 = psS.tile([P, 2 * TQ], F32, tag="s0", bufs=1)
                b1 = psS.tile([P, 2 * TQ], F32, tag="s1", bufs=1)
                qc0 = [a0[:], a1[:]]
                qc1 = [b0[:, 0:TQ], b1[:, 0:TQ]]
                _qc_head(0, qc0)
                _qc_head(1, qc1)
                _qc_tail(0, qc0)
                _qc_tail(1, qc1)
            with tc.tile_pool(name="psL", bufs=6, space="PSUM") as psL:
                for qc in range(2, NQ):
                    p0 = psL.tile([P, TQ], F32, tag="psl")
                    p1 = psL.tile([P, TQ], F32, tag="psl")
                    _qc_head(qc, [p0[:], p1[:]])
                    _qc_tail(qc, [p0[:], p1[:]])


def _build(NK=NT):
    nc = bacc.Bacc("TRN2", target_bir_lowering=False, debug=False, num_devices=8)
    dr = {}
    dr["xT"] = nc.dram_tensor("xT", [P, NC, NK * P], BF16, kind="ExternalInput")
    dr["xTq"] = nc.dram_tensor("xTq", [P, NC, TQ], BF16, kind="ExternalInput")
    for w in ("Wq", "Wk", "Wv", "Wo"):
        dr[w] = nc.dram_tensor(w, [P, NC, C], BF16, kind="ExternalInput")
    dr["vecs"] = nc.dram_tensor("vecs", [P, NC, 3], F32, kind="ExternalInput")
    dr["maskv"] = nc.dram_tensor("maskv", [P, NK], BF16, kind="ExternalInput")
    dr["boe"] = nc.dram_tensor("boe", [1, C], BF16, kind="ExternalInput")
    for v in ("lng", "lnb"):
        dr[v] = nc.dram_tensor(v, [1, C], BF16, kind="ExternalInput")
    dr["y"] = nc.dram_tensor("y", [TQ, C], BF16, kind="ExternalOutput")
    with tile.TileContext(nc) as tc:
        _emit(nc, tc, dr, NK)
    nc.compile()
    return nc


def _chunk(a):
    """[C, N] -> [128, C//128, N] with [p, c, n] = a[128c+p, n]."""
    return np.ascontiguousarray(
        a.reshape(NC, P, -1).transpose(1, 0, 2)
    )


def kernel(**inputs):
    global LAST_RESULTS
    f32 = np.float32
    Wq = np.asarray(inputs["Wq"], f32)
    Wk = np.asarray(inputs["Wk"], f32)
    Wv = np.asarray(inputs["Wv"], f32)
    Wo = np.asarray(inputs["Wo"], f32)
    x = np.asarray(inputs["x"], f32)
    mask = np.asarray(inputs["attn_mask"]).reshape(B, T)
    # sort keys so unmasked come first; masked tail chunks are dropped
    perms = [np.argsort(-mask[b], kind="stable") for b in range(B)]
    m1max = max(int(mask[b].sum()) for b in range(B))
    NK = min(NT, max(1, -(-m1max // P)))
    KL = NK * P
    key = ("nc", NK)
    if key not in _CACHE:
        _CACHE[key] = _build(NK=NK)
    nc = _CACHE[key]
    bq = np.asarray(inputs["bq"], f32)
    bk = np.asarray(inputs["bk"], f32)
    bv = np.asarray(inputs["bv"], f32)
    bo = np.asarray(inputs["bo"], f32)
    ln_g = np.asarray(inputs["ln_g"], f32)
    ln_b = np.asarray(inputs["ln_b"], f32)

    shared = {
        "Wq": _chunk(Wq).astype(NPBF16),
        "Wk": _chunk(Wk).astype(NPBF16),
        "Wv": _chunk(Wv).astype(NPBF16),
        "Wo": _chunk(Wo).astype(NPBF16),
        "boe": (bv @ Wo + bo).reshape(1, C).astype(NPBF16),
        "lng": ln_g.reshape(1, C).astype(NPBF16),
        "lnb": ln_b.reshape(1, C).astype(NPBF16),
    }
    in_maps = []
    for core in range(8):
        b, half = core // 2, core % 2
        xt = np.ascontiguousarray(x[b].T)  # [C, T]
        pk = perms[b][:KL]
        mfp = mask[b][pk].astype(f32)     # permuted/truncated key mask
        vcol = np.zeros((P, NC), f32)
        vcol[:, :NK] = mfp.reshape(NK, P).T
        vecs = np.stack([bq.reshape(NC, P).T, bk.reshape(NC, P).T, vcol], axis=-1)
        m = dict(shared)
        m["xT"] = _chunk(np.ascontiguousarray(xt[:, pk])).astype(NPBF16)
        m["xTq"] = _chunk(xt[:, half * TQ : (half + 1) * TQ]).astype(NPBF16)
        m["vecs"] = np.ascontiguousarray(vecs, f32)
        m["maskv"] = np.ascontiguousarray(mfp.reshape(NK, P).T.astype(NPBF16))
        in_maps.append(m)

    trace = os.environ.get("KERNEL_TRACE", "0") == "1"
    if trace:
        _ensure_ntff_hook()
    LAST_RESULTS = run_bass_kernel_spmd(
        nc, in_maps, core_ids=list(range(8)), trace=trace
    )
    out = np.empty((B, T, C), f32)
    for core in range(8):
        b, half = core // 2, core % 2
        out[b, half * TQ : (half + 1) * TQ, :] = LAST_RESULTS.results[core][
            "y"
        ].astype(f32)
    return out


# revision 40
# speedup vs baseline: 1.0201x; 1.0201x over previous
"""Trainium2 Bass kernel for nn_MultiHeadAttention (B=4,T=1024,C=1024,H=16).

Sharding: 8 cores = 4 batches x 2 query-halves. Each core computes, for its
batch b and its 512 query rows:
  V projection (natural layout, mask folded in, + 64 replicated mask cols per
  head so the O matmul emits the softmax denominator pre-replicated across 64
  psum partitions), then per head-pair: Q^T/K^T projection chunks,
  S^T = K^T.T @ Q^T (row-packed head pairs, D=64 contraction), E^T =
  exp(0.125*S^T) on ACT (unmasked), O^T+den via one [V_h*m | m*64] matmul,
  normalize with reciprocal_approx_fast at partition base 0 (no DRAM bounce);
  finally Y = O^T.T @ Wo + (bv@Wo+bo) (bias via K=1 matmul) and LayerNorm with
  stats on PSUM, affine on ACT, bf16 output.
Host gathers the 8 [512,1024] bf16 outputs into [4,1024,1024] f32.
"""

import os
import sys

import numpy as np

for _p in ("/opt/trn_rl_repo", "/root/.axon_site/_ro/trn_rl_repo"):
    if os.path.isdir(_p) and _p not in sys.path:
        sys.path.append(_p)

import ml_dtypes  # noqa: E402
import concourse.bass as bass  # noqa: E402
import concourse.mybir as mybir  # noqa: E402
import concourse.tile as tile  # noqa: E402
from concourse import bacc  # noqa: E402
from concourse.bass_utils import run_bass_kernel_spmd  # noqa: E402

BF16 = mybir.dt.bfloat16
F32 = mybir.dt.float32
NPBF16 = ml_dtypes.bfloat16

B, T, C, H = 4, 1024, 1024, 16
D = C // H          # 64
P = 128             # partitions
NC = C // P         # 8 chunks of C
NT = T // P         # 8 chunks of T
TQ = T // 2         # 512 query rows per core
NQ = TQ // P        # 4 query chunks
NPAIR = H // 2      # 8 head pairs
EPS = 1e-5

_CACHE = {}
LAST_RESULTS = None


def _ensure_ntff_hook():
    """Register the axon NTFF profiling hook if the image's antenv lacks it."""
    try:
        import antenv.axon_hooks  # noqa: F401
        return
    except ImportError:
        pass
    try:
        import types

        import antenv
        from trn_agent_boot.trn_boot import _ntff_profile_via_ctypes

        mod = types.ModuleType("antenv.axon_hooks")
        state = {"hook": None}
        mod.set_axon_ntff_profile_hook = lambda h: state.__setitem__("hook", h)
        mod.get_axon_ntff_profile_hook = lambda: state["hook"]
        sys.modules["antenv.axon_hooks"] = mod
        antenv.axon_hooks = mod
        hook = _ntff_profile_via_ctypes("/opt/axon/libaxon_pjrt.so")
        if hook is not None:
            mod.set_axon_ntff_profile_hook(hook)
    except Exception:
        pass


def _emit(nc, tc, dr, NK):
    """Emit the per-core Tile program."""
    from contextlib import ExitStack

    with ExitStack() as ctx:
        consts = ctx.enter_context(tc.tile_pool(name="consts", bufs=1))

        KL = NK * P
        # VA[:, jc, h, :] = [V_h*m | m x64] for even h (O rows 0:64, den64
        # rows 64:128), [m x64 | V_h*m] for odd h (den64 low, O high). The
        # mask block gives the softmax denominator replicated across 64 psum
        # partitions for free in the O matmul.
        VA = consts.tile([P, NK, H, P], BF16)
        OT = consts.tile([P, NC, TQ], BF16)        # O^T (unnormed even/odd)
        Wo_sb = consts.tile([P, NC, C], BF16)
        lng_rep = consts.tile([P, C], BF16)
        lnb_rep = consts.tile([P, C], BF16)
        boe_sb = consts.tile([1, C], BF16)         # bv@Wo + bo
        ones1 = consts.tile([1, P], BF16)
        vecs = consts.tile([P, NC, 3], F32)        # bq | bk | maskf
        maskv = consts.tile([P, NK], BF16)
        eps_t = consts.tile([P, 1], F32)

        nc.vector.memset(eps_t, EPS)
        nc.vector.memset(ones1, 1.0)
        nc.gpsimd.dma_start(out=vecs[:], in_=dr["vecs"].ap()[:])
        nc.gpsimd.dma_start(out=maskv[:], in_=dr["maskv"].ap()[:])
        nc.gpsimd.dma_start(out=boe_sb[:], in_=dr["boe"].ap()[:])

        with (
            tc.tile_pool(name="pa", bufs=1) as pa,
            tc.tile_pool(name="pb", bufs=2) as pb,
            tc.tile_pool(name="psP", bufs=2, space="PSUM") as psP,
        ):
            xT = pa.tile([P, NC, KL], BF16)
            xTq = pa.tile([P, NC, TQ], BF16)
            Wq_sb = pa.tile([P, NC, C], BF16)
            Wk_sb = pa.tile([P, NC, C], BF16)
            Wv_sb = pa.tile([P, NC, C], BF16)
            # Input loads striped across the three DMA queues (sync/scalar/
            # gpsimd each stripe over all 16 DMA engines; aggregate ~350GB/s),
            # phased by first-use: xT+Wv lo | Wv hi | xTq+Wq | Wk | Wo.
            ENG = [nc.sync, nc.scalar, nc.gpsimd]

            def _ldsplit(dst, src):
                n = dst.shape[1]
                bnd = [0, (n + 2) // 3, n - (n + 2) // 3, n]
                bnd[2] += bnd[1]
                for e in range(3):
                    lo, hi = bnd[e], bnd[e + 1]
                    if hi > lo:
                        ENG[e].dma_start(
                            out=dst[:, lo:hi], in_=src[:, lo:hi]
                        )

            # V-proj inputs interleaved per kc chunk so mm(kc) can start as
            # soon as its own xT row + Wv column block land (~10.5us).
            for kc in range(NC):
                ENG[kc % 3].dma_start(
                    out=xT[:, kc, :], in_=dr["xT"].ap()[:, kc, :]
                )
                ENG[kc % 3].dma_start(
                    out=Wv_sb[:, kc, 0:TQ], in_=dr["Wv"].ap()[:, kc, 0:TQ]
                )
            _ldsplit(Wv_sb[:, :, TQ:], dr["Wv"].ap()[:, :, TQ:])
            _ldsplit(Wk_sb[:], dr["Wk"].ap()[:])
            _ldsplit(xTq[:], dr["xTq"].ap()[:])
            _ldsplit(Wq_sb[:], dr["Wq"].ap()[:])
            _ldsplit(Wo_sb[:], dr["Wo"].ap()[:])
            for name, rep in (("lng", lng_rep), ("lnb", lnb_rep)):
                a = dr[name].ap()
                bcast = bass.AP(
                    tensor=a.tensor, offset=a.offset, ap=[[0, P], [1, C]]
                )
                nc.gpsimd.dma_start(out=rep[:], in_=bcast)

            # ---- output projection + LayerNorm body (called per qc) ----
            def _qc_head(qc, psn):
                # mc 0..6 accumulation for both halves (no reads of the last
                # pair's OT chunk) — bridge work for the tensor queue.
                qs = slice(qc * P, (qc + 1) * P)
                for nn in range(2):
                    for mc in range(NC - 1):
                        nc.tensor.matmul(
                            psn[nn],
                            OT[:, mc, qs],
                            Wo_sb[:, mc, nn * TQ : (nn + 1) * TQ],
                            start=(mc == 0),
                            stop=False,
                        )

            def _qc_tail(qc, psn):
                qs = slice(qc * P, (qc + 1) * P)
                for nn in range(2):
                    nc.tensor.matmul(
                        psn[nn],
                        OT[:, NC - 1, qs],
                        Wo_sb[:, NC - 1, nn * TQ : (nn + 1) * TQ],
                        start=False, stop=False,
                    )
                    nc.tensor.matmul(
                        psn[nn],
                        ones1[0:1, :],
                        boe_sb[0:1, nn * TQ : (nn + 1) * TQ],
                        start=False, stop=True,
                    )
                stats = pb.tile(
                    [P, 2, nc.vector.BN_STATS_DIM], F32, tag="stats"
                )
                mv = pb.tile([P, nc.vector.BN_AGGR_DIM], F32, tag="mv")
                nc.vector.bn_stats(out=stats[:, 0, :], in_=psn[0][:])
                nc.vector.bn_stats(out=stats[:, 1, :], in_=psn[1][:])
                nc.vector.bn_aggr(out=mv[:], in_=stats[:])
                rstd = pb.tile([P, 1], F32, tag="rstd")
                nmr = pb.tile([P, 1], F32, tag="nmr")
                nc.scalar.activation(
                    out=rstd[:], in_=mv[:, 1:2],
                    func=mybir.ActivationFunctionType.Sqrt,
                    bias=eps_t[:], scale=1.0,
                )
                nc.vector.reciprocal(out=rstd[:], in_=rstd[:])
                nc.vector.tensor_scalar(
                    nmr[:], mv[:, 0:1], rstd[:], -1.0,
                    mybir.AluOpType.mult, mybir.AluOpType.mult,
                )
                Yf = pb.tile([P, C], BF16, tag="yf")
                for nn in range(2):
                    sl = slice(nn * TQ, (nn + 1) * TQ)
                    nc.scalar.activation(
                        out=Yf[:, sl], in_=psn[nn][:],
                        func=mybir.ActivationFunctionType.Identity,
                        bias=nmr[:], scale=rstd[:],
                    )
                    nc.vector.tensor_tensor(
                        Yf[:, sl], Yf[:, sl], lng_rep[:, sl],
                        mybir.AluOpType.mult,
                    )
                    nc.vector.tensor_tensor(
                        Yf[:, sl], Yf[:, sl], lnb_rep[:, sl],
                        mybir.AluOpType.add,
                    )
                    (nc.sync if nn == 0 else nc.gpsimd).dma_start(
                        out=dr["y"].ap()[qs, sl], in_=Yf[:, sl]
                    )

            with (
                tc.tile_pool(name="psS", bufs=2, space="PSUM") as psS,
                tc.tile_pool(name="psO", bufs=2, space="PSUM") as psO,
            ):
                # ---- V projection: [keys, C] natural, masked, + mask block --
                for nn in range(2):
                    for tcn in range(NK):
                        ps = psP.tile([P, TQ], F32, tag="psp")
                        for kc in range(NC):
                            nc.tensor.matmul(
                                ps[:],
                                xT[:, kc, tcn * P : (tcn + 1) * P],
                                Wv_sb[:, kc, nn * TQ : (nn + 1) * TQ],
                                start=(kc == 0),
                                stop=(kc == NC - 1),
                            )
                        # V blocks land at +0 (even heads) / +64 (odd heads)
                        a = VA[:, tcn, :, :]
                        vdst = bass.AP(
                            tensor=a.tensor, offset=a.offset + nn * 8 * P,
                            ap=[a.ap[0], [2 * P, 4], [P + D, 2], [1, D]],
                        )
                        nc.vector.tensor_scalar_mul(
                            vdst,
                            ps[:].rearrange("p (a b d) -> p a b d", a=4, b=2),
                            vecs[:, tcn, 2:3],
                        )
                        if nn == 0:
                            # mask blocks: +64 for even heads, +0 for odd
                            mdst = bass.AP(
                                tensor=a.tensor, offset=a.offset + D,
                                ap=[a.ap[0], [2 * P, 8], [D, 2], [1, D]],
                            )
                            nc.vector.tensor_copy(
                                out=mdst,
                                in_=maskv[:, tcn, None].to_broadcast(
                                    (P, 8, 2, D)
                                ),
                            )

                # ---- per head-pair: QT/KT proj, S^T, exp, O^T, normalize,
                # software-pipelined so K/Q(c+1) matmuls cover exp(c) on ACT.
                def emit_kq(c):
                    QTc = pb.tile([P, TQ], BF16, tag="qtc")
                    KTc = pb.tile([P, KL], BF16, tag="ktc")
                    for ko in range(0, KL, TQ):
                        w = min(TQ, KL - ko)
                        ps = psP.tile([P, TQ], F32, tag="psp")
                        for kc in range(NC):
                            nc.tensor.matmul(
                                ps[:, :w],
                                Wk_sb[:, kc, c * P : (c + 1) * P],
                                xT[:, kc, ko : ko + w],
                                start=(kc == 0),
                                stop=(kc == NC - 1),
                            )
                        nc.vector.tensor_scalar_add(
                            KTc[:, ko : ko + w], ps[:, :w], vecs[:, c, 1:2]
                        )
                    ps = psP.tile([P, TQ], F32, tag="psp")
                    for kc in range(NC):
                        nc.tensor.matmul(
                            ps[:],
                            Wq_sb[:, kc, c * P : (c + 1) * P],
                            xTq[:, kc, :],
                            start=(kc == 0),
                            stop=(kc == NC - 1),
                        )
                    nc.vector.tensor_scalar_add(QTc[:], ps[:], vecs[:, c, 0:1])
                    return QTc, KTc

                def emit_s(c, QTc, KTc):
                    et0 = pb.tile([P, NK, TQ], BF16, tag="et0", bufs=2)
                    et1 = pb.tile([P, NK, TQ], BF16, tag="et1", bufs=2)
                    for jj in range((NK + 1) // 2):
                        nu = min(2, NK - 2 * jj)
                        s0 = psS.tile([P, 2 * TQ], F32, tag="s0", bufs=1)
                        s1 = psS.tile([P, 2 * TQ], F32, tag="s1", bufs=1)
                        for u in range(nu):
                            jc = 2 * jj + u
                            js = slice(jc * P, (jc + 1) * P)
                            nc.tensor.matmul(
                                s0[:, u * TQ : (u + 1) * TQ],
                                KTc[0:D, js],
                                QTc[0:D, :],
                                start=True, stop=True,
                                tile_position=(0, 0),
                            )
                            nc.tensor.matmul(
                                s1[:, u * TQ : (u + 1) * TQ],
                                KTc[D:P, js],
                                QTc[D:P, :],
                                start=True, stop=True,
                                tile_position=(D, 0),
                            )
                        nc.scalar.activation(
                            out=et0[:, 2 * jj : 2 * jj + nu, :],
                            in_=s0[:, : nu * TQ],
                            func=mybir.ActivationFunctionType.Exp, scale=0.125,
                        )
                        nc.scalar.activation(
                            out=et1[:, 2 * jj : 2 * jj + nu, :],
                            in_=s1[:, : nu * TQ],
                            func=mybir.ActivationFunctionType.Exp, scale=0.125,
                        )
                    return et0, et1

                def emit_o_den(c, et0, et1):
                    h0, h1 = 2 * c, 2 * c + 1
                    # O^T + den64: even head [O | den64], odd [den64 | O].
                    # The even head's den chain (ACT copy -> DMA shift ->
                    # recip) is emitted between the two O matmul groups so it
                    # overlaps the odd head's matmuls.
                    po0 = psO.tile([P, TQ], F32, tag="po0", bufs=1)
                    po1 = psO.tile([P, TQ], F32, tag="po1", bufs=1)
                    dt = pb.tile([P, 2, TQ], F32, tag="dt")
                    rr = pb.tile([P, 2, TQ], F32, tag="rr")
                    for jc in range(NK):
                        nc.tensor.matmul(
                            po0[:], VA[:, jc, h0, :], et0[:, jc, :],
                            start=(jc == 0), stop=(jc == NK - 1),
                        )
                    nc.scalar.activation(
                        out=dt[D:P, 0, :], in_=po0[D:P, :],
                        func=mybir.ActivationFunctionType.Identity,
                    )
                    nc.sync.dma_start(out=dt[0:D, 0, :], in_=dt[D:P, 0, :])
                    nc.vector.reciprocal_approx_fast(
                        out=rr[0:D, 0, :], in_=dt[0:D, 0, :]
                    )
                    for jc in range(NK):
                        nc.tensor.matmul(
                            po1[:], VA[:, jc, h1, :], et1[:, jc, :],
                            start=(jc == 0), stop=(jc == NK - 1),
                        )
                    nc.vector.reciprocal_approx_fast(
                        out=rr[0:D, 1, :], in_=po1[0:D, :]
                    )
                    nc.sync.dma_start(out=rr[D:P, 1, :], in_=rr[0:D, 1, :])
                    nc.vector.tensor_tensor(
                        OT[0:D, c, :], po0[0:D, :], rr[0:D, 0, :],
                        mybir.AluOpType.mult,
                    )
                    nc.vector.tensor_tensor(
                        OT[D:P, c, :], po1[D:P, :], rr[D:P, 1, :],
                        mybir.AluOpType.mult,
                    )

                ets = emit_s(0, *emit_kq(0))
                for c in range(NPAIR):
                    nxt = emit_kq(c + 1) if c + 1 < NPAIR else None
                    emit_o_den(c, *ets)
                    if nxt is not None:
                        ets = emit_s(c + 1, *nxt)

                # qc 0/1 emitted inside the psS/psO scope from psP so their
                # matmuls bridge the pool-close barrier (which waits on the
                # last pair's den chain); the rest pipeline deeply from psL.
                # Bridge the pool-close barrier: four mc0..6 chains emitted
                # before any mc7 tail (the in-order tensor queue would
                # otherwise block on the last pair's OT chunk). qc1's psums
                # reuse the psS slots, free right after pair 7's exp.
                a0 = psP.tile([P, TQ], F32, tag="psp")
                a1 = psP.tile([P, TQ], F32, tag="psp")
                b0 = psO.tile([P, TQ], F32, tag="po0", bufs=1)
                b1 = psO.tile([P, TQ], F32, tag="po1", bufs=1)
                qc0 = [a0[:], a1[:]]
                qc1 = [b0[:], b1[:]]
                _qc_head(0, qc0)
                _qc_head(1, qc1)
                _qc_tail(0, qc0)
                _qc_tail(1, qc1)
            with tc.tile_pool(name="psL", bufs=4, space="PSUM") as psL:
                for qc in range(2, NQ):
                    p0 = psL.tile([P, TQ], F32, tag="psl")
                    p1 = psL.tile([P, TQ], F32, tag="psl")
                    _qc_head(qc, [p0[:], p1[:]])
                    _qc_tail(qc, [p0[:], p1[:]])


def _build(NK=NT):
    nc = bacc.Bacc("TRN2", target_bir_lowering=False, debug=False, num_devices=8)
    dr = {}
    dr["xT"] = nc.dram_tensor("xT", [P, NC, NK * P], BF16, kind="ExternalInput")
    dr["xTq"] = nc.dram_tensor("xTq", [P, NC, TQ], BF16, kind="ExternalInput")
    for w in ("Wq", "Wk", "Wv", "Wo"):
        dr[w] = nc.dram_tensor(w, [P, NC, C], BF16, kind="ExternalInput")
    dr["vecs"] = nc.dram_tensor("vecs", [P, NC, 3], F32, kind="ExternalInput")
    dr["maskv"] = nc.dram_tensor("maskv", [P, NK], BF16, kind="ExternalInput")
    dr["boe"] = nc.dram_tensor("boe", [1, C], BF16, kind="ExternalInput")
    for v in ("lng", "lnb"):
        dr[v] = nc.dram_tensor(v, [1, C], BF16, kind="ExternalInput")
    dr["y"] = nc.dram_tensor("y", [TQ, C], BF16, kind="ExternalOutput")
    with tile.TileContext(nc) as tc:
        _emit(nc, tc, dr, NK)
    nc.compile()
    return nc


def _chunk(a):
    """[C, N] -> [128, C//128, N] with [p, c, n] = a[128c+p, n]."""
    return np.ascontiguousarray(
        a.reshape(NC, P, -1).transpose(1, 0, 2)
    )


def kernel(**inputs):
    global LAST_RESULTS
    f32 = np.float32
    Wq = np.asarray(inputs["Wq"], f32)
    Wk = np.asarray(inputs["Wk"], f32)
    Wv = np.asarray(inputs["Wv"], f32)
    Wo = np.asarray(inputs["Wo"], f32)
    x = np.asarray(inputs["x"], f32)
    mask = np.asarray(inputs["attn_mask"]).reshape(B, T)
    # sort keys so unmasked come first; masked tail chunks are dropped
    perms = [np.argsort(-mask[b], kind="stable") for b in range(B)]
    m1max = max(int(mask[b].sum()) for b in range(B))
    NK = min(NT, max(1, -(-m1max // P)))
    KL = NK * P
    key = ("nc", NK)
    if key not in _CACHE:
        _CACHE[key] = _build(NK=NK)
    nc = _CACHE[key]
    bq = np.asarray(inputs["bq"], f32)
    bk = np.asarray(inputs["bk"], f32)
    bv = np.asarray(inputs["bv"], f32)
    bo = np.asarray(inputs["bo"], f32)
    ln_g = np.asarray(inputs["ln_g"], f32)
    ln_b = np.asarray(inputs["ln_b"], f32)

    shared = {
        "Wq": _chunk(Wq).astype(NPBF16),
        "Wk": _chunk(Wk).astype(NPBF16),
        "Wv": _chunk(Wv).astype(NPBF16),
        "Wo": _chunk(Wo).astype(NPBF16),
        "boe": (bv @ Wo + bo).reshape(1, C).astype(NPBF16),
        "lng": ln_g.reshape(1, C).astype(NPBF16),
        "lnb": ln_b.reshape(1, C).astype(NPBF16),
    }
    in_maps = []
    for core in range(8):
        b, half = core // 2, core % 2
        xt = np.ascontiguousarray(x[b].T)  # [C, T]
        pk = perms[b][:KL]
        mfp = mask[b][pk].astype(f32)     # permuted/truncated key mask
        vcol = np.zeros((P, NC), f32)
        vcol[:, :NK] = mfp.reshape(NK, P).T
        vecs = np.stack([bq.reshape(NC, P).T, bk.reshape(NC, P).T, vcol], axis=-1)
        m = dict(shared)
        m["xT"] = _chunk(np.ascontiguousarray(xt[:, pk])).astype(NPBF16)
        m["xTq"] = _chunk(xt[:, half * TQ : (half + 1) * TQ]).astype(NPBF16)
        m["vecs"] = np.ascontiguousarray(vecs, f32)
        m["maskv"] = np.ascontiguousarray(mfp.reshape(NK, P).T.astype(NPBF16))
        in_maps.append(m)

    trace = os.environ.get("KERNEL_TRACE", "0") == "1"
    if trace:
        _ensure_ntff_hook()
    LAST_RESULTS = run_bass_kernel_spmd(
        nc, in_maps, core_ids=list(range(8)), trace=trace
    )
    out = np.empty((B, T, C), f32)
    for core in range(8):
        b, half = core // 2, core % 2
        out[b, half * TQ : (half + 1) * TQ, :] = LAST_RESULTS.results[core][
            "y"
        ].astype(f32)
    return out


# revision 41
# speedup vs baseline: 1.0288x; 1.0085x over previous
"""Trainium2 Bass kernel for nn_MultiHeadAttention (B=4,T=1024,C=1024,H=16).

Sharding: 8 cores = 4 batches x 2 query-halves. Each core computes, for its
batch b and its 512 query rows:
  V projection (natural layout, mask folded in, + 64 replicated mask cols per
  head so the O matmul emits the softmax denominator pre-replicated across 64
  psum partitions), then per head-pair: Q^T/K^T projection chunks,
  S^T = K^T.T @ Q^T (row-packed head pairs, D=64 contraction), E^T =
  exp(0.125*S^T) on ACT (unmasked), O^T+den via one [V_h*m | m*64] matmul,
  normalize with reciprocal_approx_fast at partition base 0 (no DRAM bounce);
  finally Y = O^T.T @ Wo + (bv@Wo+bo) (bias via K=1 matmul) and LayerNorm with
  stats on PSUM, affine on ACT, bf16 output.
Host gathers the 8 [512,1024] bf16 outputs into [4,1024,1024] f32.
"""

import os
import sys

import numpy as np

for _p in ("/opt/trn_rl_repo", "/root/.axon_site/_ro/trn_rl_repo"):
    if os.path.isdir(_p) and _p not in sys.path:
        sys.path.append(_p)

import ml_dtypes  # noqa: E402
import concourse.bass as bass  # noqa: E402
import concourse.mybir as mybir  # noqa: E402
import concourse.tile as tile  # noqa: E402
from concourse import bacc  # noqa: E402
from concourse.bass_utils import run_bass_kernel_spmd  # noqa: E402

BF16 = mybir.dt.bfloat16
F32 = mybir.dt.float32
NPBF16 = ml_dtypes.bfloat16

B, T, C, H = 4, 1024, 1024, 16
D = C // H          # 64
P = 128             # partitions
NC = C // P         # 8 chunks of C
NT = T // P         # 8 chunks of T
TQ = T // 2         # 512 query rows per core
NQ = TQ // P        # 4 query chunks
NPAIR = H // 2      # 8 head pairs
EPS = 1e-5

_CACHE = {}
LAST_RESULTS = None


def _ensure_ntff_hook():
    """Register the axon NTFF profiling hook if the image's antenv lacks it."""
    try:
        import antenv.axon_hooks  # noqa: F401
        return
    except ImportError:
        pass
    try:
        import types

        import antenv
        from trn_agent_boot.trn_boot import _ntff_profile_via_ctypes

        mod = types.ModuleType("antenv.axon_hooks")
        state = {"hook": None}
        mod.set_axon_ntff_profile_hook = lambda h: state.__setitem__("hook", h)
        mod.get_axon_ntff_profile_hook = lambda: state["hook"]
        sys.modules["antenv.axon_hooks"] = mod
        antenv.axon_hooks = mod
        hook = _ntff_profile_via_ctypes("/opt/axon/libaxon_pjrt.so")
        if hook is not None:
            mod.set_axon_ntff_profile_hook(hook)
    except Exception:
        pass


def _emit(nc, tc, dr, NK):
    """Emit the per-core Tile program."""
    from contextlib import ExitStack

    with ExitStack() as ctx:
        consts = ctx.enter_context(tc.tile_pool(name="consts", bufs=1))

        KL = NK * P
        # VA[:, jc, h, :] = [V_h*m | m x64] for even h (O rows 0:64, den64
        # rows 64:128), [m x64 | V_h*m] for odd h (den64 low, O high). The
        # mask block gives the softmax denominator replicated across 64 psum
        # partitions for free in the O matmul.
        VA = consts.tile([P, NK, H, P], BF16)
        OT = consts.tile([P, NC, TQ], BF16)        # O^T (unnormed even/odd)
        Wo_sb = consts.tile([P, NC, C], BF16)
        lng_rep = consts.tile([P, C], BF16)
        lnb_rep = consts.tile([P, C], BF16)
        boe_sb = consts.tile([1, C], BF16)         # bv@Wo + bo
        ones1 = consts.tile([1, P], BF16)
        vecs = consts.tile([P, NC, 3], F32)        # bq | bk | maskf
        maskv = consts.tile([P, NK], BF16)
        eps_t = consts.tile([P, 1], F32)

        nc.vector.memset(eps_t, EPS)
        nc.vector.memset(ones1, 1.0)
        nc.gpsimd.dma_start(out=vecs[:], in_=dr["vecs"].ap()[:])
        nc.gpsimd.dma_start(out=maskv[:], in_=dr["maskv"].ap()[:])
        nc.gpsimd.dma_start(out=boe_sb[:], in_=dr["boe"].ap()[:])

        with (
            tc.tile_pool(name="pa", bufs=1) as pa,
            tc.tile_pool(name="pb", bufs=2) as pb,
            tc.tile_pool(name="psP", bufs=2, space="PSUM") as psP,
        ):
            xT = pa.tile([P, NC, KL], BF16)
            xTq = pa.tile([P, NC, TQ], BF16)
            Wq_sb = pa.tile([P, NC, C], BF16)
            Wk_sb = pa.tile([P, NC, C], BF16)
            Wv_sb = pa.tile([P, NC, C], BF16)
            # Input loads striped across the three DMA queues (sync/scalar/
            # gpsimd each stripe over all 16 DMA engines; aggregate ~350GB/s),
            # phased by first-use: xT+Wv lo | Wv hi | xTq+Wq | Wk | Wo.
            ENG = [nc.sync, nc.scalar, nc.gpsimd]

            def _ldsplit(dst, src):
                n = dst.shape[1]
                bnd = [0, (n + 2) // 3, n - (n + 2) // 3, n]
                bnd[2] += bnd[1]
                for e in range(3):
                    lo, hi = bnd[e], bnd[e + 1]
                    if hi > lo:
                        ENG[e].dma_start(
                            out=dst[:, lo:hi], in_=src[:, lo:hi]
                        )

            # V-proj inputs interleaved per kc chunk so mm(kc) can start as
            # soon as its own xT row + Wv column block land (~10.5us).
            for kc in range(NC):
                ENG[kc % 3].dma_start(
                    out=xT[:, kc, :], in_=dr["xT"].ap()[:, kc, :]
                )
                ENG[kc % 3].dma_start(
                    out=Wv_sb[:, kc, 0:TQ], in_=dr["Wv"].ap()[:, kc, 0:TQ]
                )
            _ldsplit(Wv_sb[:, :, TQ:], dr["Wv"].ap()[:, :, TQ:])
            _ldsplit(Wk_sb[:], dr["Wk"].ap()[:])
            _ldsplit(xTq[:], dr["xTq"].ap()[:])
            _ldsplit(Wq_sb[:], dr["Wq"].ap()[:])
            _ldsplit(Wo_sb[:], dr["Wo"].ap()[:])
            for name, rep in (("lng", lng_rep), ("lnb", lnb_rep)):
                a = dr[name].ap()
                bcast = bass.AP(
                    tensor=a.tensor, offset=a.offset, ap=[[0, P], [1, C]]
                )
                nc.gpsimd.dma_start(out=rep[:], in_=bcast)

            # ---- output projection + LayerNorm body (called per qc) ----
            def _qc_head(qc, psn):
                # mc 0..6 accumulation for both halves (no reads of the last
                # pair's OT chunk) — bridge work for the tensor queue.
                qs = slice(qc * P, (qc + 1) * P)
                for nn in range(2):
                    for mc in range(NC - 1):
                        nc.tensor.matmul(
                            psn[nn],
                            OT[:, mc, qs],
                            Wo_sb[:, mc, nn * TQ : (nn + 1) * TQ],
                            start=(mc == 0),
                            stop=False,
                        )

            def _qc_tail(qc, psn):
                qs = slice(qc * P, (qc + 1) * P)
                for nn in range(2):
                    nc.tensor.matmul(
                        psn[nn],
                        OT[:, NC - 1, qs],
                        Wo_sb[:, NC - 1, nn * TQ : (nn + 1) * TQ],
                        start=False, stop=False,
                    )
                    nc.tensor.matmul(
                        psn[nn],
                        ones1[0:1, :],
                        boe_sb[0:1, nn * TQ : (nn + 1) * TQ],
                        start=False, stop=True,
                    )
                stats = pb.tile(
                    [P, 2, nc.vector.BN_STATS_DIM], F32, tag="stats"
                )
                mv = pb.tile([P, nc.vector.BN_AGGR_DIM], F32, tag="mv")
                nc.vector.bn_stats(out=stats[:, 0, :], in_=psn[0][:])
                nc.vector.bn_stats(out=stats[:, 1, :], in_=psn[1][:])
                nc.vector.bn_aggr(out=mv[:], in_=stats[:])
                rstd = pb.tile([P, 1], F32, tag="rstd")
                nmr = pb.tile([P, 1], F32, tag="nmr")
                nc.scalar.activation(
                    out=rstd[:], in_=mv[:, 1:2],
                    func=mybir.ActivationFunctionType.Sqrt,
                    bias=eps_t[:], scale=1.0,
                )
                nc.vector.reciprocal(out=rstd[:], in_=rstd[:])
                nc.vector.tensor_scalar(
                    nmr[:], mv[:, 0:1], rstd[:], -1.0,
                    mybir.AluOpType.mult, mybir.AluOpType.mult,
                )
                Yf = pb.tile([P, C], BF16, tag="yf")
                for nn in range(2):
                    sl = slice(nn * TQ, (nn + 1) * TQ)
                    nc.scalar.activation(
                        out=Yf[:, sl], in_=psn[nn][:],
                        func=mybir.ActivationFunctionType.Identity,
                        bias=nmr[:], scale=rstd[:],
                    )
                    nc.vector.tensor_tensor(
                        Yf[:, sl], Yf[:, sl], lng_rep[:, sl],
                        mybir.AluOpType.mult,
                    )
                    nc.vector.tensor_tensor(
                        Yf[:, sl], Yf[:, sl], lnb_rep[:, sl],
                        mybir.AluOpType.add,
                    )
                    (nc.sync if nn == 0 else nc.gpsimd).dma_start(
                        out=dr["y"].ap()[qs, sl], in_=Yf[:, sl]
                    )

            with (
                tc.tile_pool(name="psS", bufs=2, space="PSUM") as psS,
                tc.tile_pool(name="psO", bufs=2, space="PSUM") as psO,
            ):
                # ---- V projection: [keys, C] natural, masked, + mask block --
                for nn in range(2):
                    for tcn in range(NK):
                        ps = psP.tile([P, TQ], F32, tag="psp")
                        for kc in range(NC):
                            nc.tensor.matmul(
                                ps[:],
                                xT[:, kc, tcn * P : (tcn + 1) * P],
                                Wv_sb[:, kc, nn * TQ : (nn + 1) * TQ],
                                start=(kc == 0),
                                stop=(kc == NC - 1),
                            )
                        # V blocks land at +0 (even heads) / +64 (odd heads)
                        a = VA[:, tcn, :, :]
                        vdst = bass.AP(
                            tensor=a.tensor, offset=a.offset + nn * 8 * P,
                            ap=[a.ap[0], [2 * P, 4], [P + D, 2], [1, D]],
                        )
                        nc.vector.tensor_scalar_mul(
                            vdst,
                            ps[:].rearrange("p (a b d) -> p a b d", a=4, b=2),
                            vecs[:, tcn, 2:3],
                        )
                        if nn == 0:
                            # mask blocks: +64 for even heads, +0 for odd
                            mdst = bass.AP(
                                tensor=a.tensor, offset=a.offset + D,
                                ap=[a.ap[0], [2 * P, 8], [D, 2], [1, D]],
                            )
                            nc.vector.tensor_copy(
                                out=mdst,
                                in_=maskv[:, tcn, None].to_broadcast(
                                    (P, 8, 2, D)
                                ),
                            )

                # ---- per head-pair: QT/KT proj, S^T, exp, O^T, normalize,
                # software-pipelined so K/Q(c+1) matmuls cover exp(c) on ACT.
                def emit_kq(c):
                    QTc = pb.tile([P, TQ], BF16, tag="qtc")
                    KTc = pb.tile([P, KL], BF16, tag="ktc")
                    for ko in range(0, KL, TQ):
                        w = min(TQ, KL - ko)
                        ps = psP.tile([P, TQ], F32, tag="psp")
                        for kc in range(NC):
                            nc.tensor.matmul(
                                ps[:, :w],
                                Wk_sb[:, kc, c * P : (c + 1) * P],
                                xT[:, kc, ko : ko + w],
                                start=(kc == 0),
                                stop=(kc == NC - 1),
                            )
                        nc.vector.tensor_scalar_add(
                            KTc[:, ko : ko + w], ps[:, :w], vecs[:, c, 1:2]
                        )
                    ps = psP.tile([P, TQ], F32, tag="psp")
                    for kc in range(NC):
                        nc.tensor.matmul(
                            ps[:],
                            Wq_sb[:, kc, c * P : (c + 1) * P],
                            xTq[:, kc, :],
                            start=(kc == 0),
                            stop=(kc == NC - 1),
                        )
                    nc.vector.tensor_scalar_add(QTc[:], ps[:], vecs[:, c, 0:1])
                    return QTc, KTc

                def emit_s(c, QTc, KTc):
                    et0 = pb.tile([P, NK, TQ], BF16, tag="et0", bufs=2)
                    et1 = pb.tile([P, NK, TQ], BF16, tag="et1", bufs=2)
                    for jj in range((NK + 1) // 2):
                        nu = min(2, NK - 2 * jj)
                        s0 = psS.tile([P, 2 * TQ], F32, tag="s0", bufs=1)
                        s1 = psS.tile([P, 2 * TQ], F32, tag="s1", bufs=1)
                        for u in range(nu):
                            jc = 2 * jj + u
                            js = slice(jc * P, (jc + 1) * P)
                            nc.tensor.matmul(
                                s0[:, u * TQ : (u + 1) * TQ],
                                KTc[0:D, js],
                                QTc[0:D, :],
                                start=True, stop=True,
                                tile_position=(0, 0),
                            )
                            nc.tensor.matmul(
                                s1[:, u * TQ : (u + 1) * TQ],
                                KTc[D:P, js],
                                QTc[D:P, :],
                                start=True, stop=True,
                                tile_position=(D, 0),
                            )
                        nc.scalar.activation(
                            out=et0[:, 2 * jj : 2 * jj + nu, :],
                            in_=s0[:, : nu * TQ],
                            func=mybir.ActivationFunctionType.Exp, scale=0.125,
                        )
                        nc.scalar.activation(
                            out=et1[:, 2 * jj : 2 * jj + nu, :],
                            in_=s1[:, : nu * TQ],
                            func=mybir.ActivationFunctionType.Exp, scale=0.125,
                        )
                    return et0, et1

                def emit_o_den(c, et0, et1):
                    h0, h1 = 2 * c, 2 * c + 1
                    # O^T + den64: even head [O | den64], odd [den64 | O].
                    # The even head's den chain (ACT copy -> DMA shift ->
                    # recip) is emitted between the two O matmul groups so it
                    # overlaps the odd head's matmuls.
                    po0 = psO.tile([P, TQ], F32, tag="po0", bufs=1)
                    po1 = psO.tile([P, TQ], F32, tag="po1", bufs=1)
                    dt = pb.tile([P, 2, TQ], F32, tag="dt")
                    rr = pb.tile([P, 2, TQ], F32, tag="rr")
                    for jc in range(NK):
                        nc.tensor.matmul(
                            po0[:], VA[:, jc, h0, :], et0[:, jc, :],
                            start=(jc == 0), stop=(jc == NK - 1),
                        )
                    nc.scalar.activation(
                        out=dt[D:P, 0, :], in_=po0[D:P, :],
                        func=mybir.ActivationFunctionType.Identity,
                    )
                    nc.sync.dma_start(out=dt[0:D, 0, :], in_=dt[D:P, 0, :])
                    nc.vector.reciprocal_approx_fast(
                        out=rr[0:D, 0, :], in_=dt[0:D, 0, :]
                    )
                    for jc in range(NK):
                        nc.tensor.matmul(
                            po1[:], VA[:, jc, h1, :], et1[:, jc, :],
                            start=(jc == 0), stop=(jc == NK - 1),
                        )
                    nc.vector.reciprocal_approx_fast(
                        out=rr[0:D, 1, :], in_=po1[0:D, :]
                    )
                    nc.sync.dma_start(out=rr[D:P, 1, :], in_=rr[0:D, 1, :])
                    nc.vector.tensor_tensor(
                        OT[0:D, c, :], po0[0:D, :], rr[0:D, 0, :],
                        mybir.AluOpType.mult,
                    )
                    nc.vector.tensor_tensor(
                        OT[D:P, c, :], po1[D:P, :], rr[D:P, 1, :],
                        mybir.AluOpType.mult,
                    )

                ets = emit_s(0, *emit_kq(0))
                for c in range(NPAIR):
                    nxt = emit_kq(c + 1) if c + 1 < NPAIR else None
                    emit_o_den(c, *ets)
                    if nxt is not None:
                        ets = emit_s(c + 1, *nxt)

                # qc 0/1 emitted inside the psS/psO scope from psP so their
                # matmuls bridge the pool-close barrier (which waits on the
                # last pair's den chain); the rest pipeline deeply from psL.
                # Bridge the pool-close barrier: four mc0..6 chains emitted
                # before any mc7 tail (the in-order tensor queue would
                # otherwise block on the last pair's OT chunk). qc1's psums
                # reuse the psS slots, free right after pair 7's exp.
                a0 = psP.tile([P, TQ], F32, tag="psp")
                a1 = psP.tile([P, TQ], F32, tag="psp")
                b0 = psO.tile([P, TQ], F32, tag="po0", bufs=1)
                b1 = psO.tile([P, TQ], F32, tag="po1", bufs=1)
                qc0 = [a0[:], a1[:]]
                qc1 = [b0[:], b1[:]]
                _qc_head(0, qc0)
                _qc_head(1, qc1)
                _qc_tail(0, qc0)
                _qc_tail(1, qc1)
            with tc.tile_pool(name="psL", bufs=4, space="PSUM") as psL:
                for qc in range(2, NQ):
                    p0 = psL.tile([P, TQ], F32, tag="psl")
                    p1 = psL.tile([P, TQ], F32, tag="psl")
                    psn = [p0[:], p1[:]]
                    if qc < NQ - 1:
                        _qc_head(qc, psn)
                        _qc_tail(qc, psn)
                        continue
                    # last qc: finish half 0 completely first so its
                    # bn_stats overlaps half 1's matmuls, shortening the
                    # exposed end-of-kernel LN chain.
                    qs = slice(qc * P, (qc + 1) * P)
                    stats = pb.tile(
                        [P, 2, nc.vector.BN_STATS_DIM], F32, tag="stats"
                    )
                    for nn in range(2):
                        for mc in range(NC):
                            nc.tensor.matmul(
                                psn[nn],
                                OT[:, mc, qs],
                                Wo_sb[:, mc, nn * TQ : (nn + 1) * TQ],
                                start=(mc == 0),
                                stop=False,
                            )
                        nc.tensor.matmul(
                            psn[nn],
                            ones1[0:1, :],
                            boe_sb[0:1, nn * TQ : (nn + 1) * TQ],
                            start=False, stop=True,
                        )
                        nc.vector.bn_stats(out=stats[:, nn, :], in_=psn[nn])
                    mv = pb.tile([P, nc.vector.BN_AGGR_DIM], F32, tag="mv")
                    nc.vector.bn_aggr(out=mv[:], in_=stats[:])
                    rstd = pb.tile([P, 1], F32, tag="rstd")
                    nmr = pb.tile([P, 1], F32, tag="nmr")
                    nc.scalar.activation(
                        out=rstd[:], in_=mv[:, 1:2],
                        func=mybir.ActivationFunctionType.Sqrt,
                        bias=eps_t[:], scale=1.0,
                    )
                    nc.vector.reciprocal(out=rstd[:], in_=rstd[:])
                    nc.vector.tensor_scalar(
                        nmr[:], mv[:, 0:1], rstd[:], -1.0,
                        mybir.AluOpType.mult, mybir.AluOpType.mult,
                    )
                    Yf = pb.tile([P, C], BF16, tag="yf")
                    for nn in range(2):
                        sl = slice(nn * TQ, (nn + 1) * TQ)
                        nc.scalar.activation(
                            out=Yf[:, sl], in_=psn[nn],
                            func=mybir.ActivationFunctionType.Identity,
                            bias=nmr[:], scale=rstd[:],
                        )
                        nc.vector.tensor_tensor(
                            Yf[:, sl], Yf[:, sl], lng_rep[:, sl],
                            mybir.AluOpType.mult,
                        )
                        nc.vector.tensor_tensor(
                            Yf[:, sl], Yf[:, sl], lnb_rep[:, sl],
                            mybir.AluOpType.add,
                        )
                        (nc.sync if nn == 0 else nc.gpsimd).dma_start(
                            out=dr["y"].ap()[qs, sl], in_=Yf[:, sl]
                        )


def _build(NK=NT):
    nc = bacc.Bacc("TRN2", target_bir_lowering=False, debug=False, num_devices=8)
    dr = {}
    dr["xT"] = nc.dram_tensor("xT", [P, NC, NK * P], BF16, kind="ExternalInput")
    dr["xTq"] = nc.dram_tensor("xTq", [P, NC, TQ], BF16, kind="ExternalInput")
    for w in ("Wq", "Wk", "Wv", "Wo"):
        dr[w] = nc.dram_tensor(w, [P, NC, C], BF16, kind="ExternalInput")
    dr["vecs"] = nc.dram_tensor("vecs", [P, NC, 3], F32, kind="ExternalInput")
    dr["maskv"] = nc.dram_tensor("maskv", [P, NK], BF16, kind="ExternalInput")
    dr["boe"] = nc.dram_tensor("boe", [1, C], BF16, kind="ExternalInput")
    for v in ("lng", "lnb"):
        dr[v] = nc.dram_tensor(v, [1, C], BF16, kind="ExternalInput")
    dr["y"] = nc.dram_tensor("y", [TQ, C], BF16, kind="ExternalOutput")
    with tile.TileContext(nc) as tc:
        _emit(nc, tc, dr, NK)
    nc.compile()
    return nc


def _chunk(a):
    """[C, N] -> [128, C//128, N] with [p, c, n] = a[128c+p, n]."""
    return np.ascontiguousarray(
        a.reshape(NC, P, -1).transpose(1, 0, 2)
    )


def kernel(**inputs):
    global LAST_RESULTS
    f32 = np.float32
    Wq = np.asarray(inputs["Wq"], f32)
    Wk = np.asarray(inputs["Wk"], f32)
    Wv = np.asarray(inputs["Wv"], f32)
    Wo = np.asarray(inputs["Wo"], f32)
    x = np.asarray(inputs["x"], f32)
    mask = np.asarray(inputs["attn_mask"]).reshape(B, T)
    # sort keys so unmasked come first; masked tail chunks are dropped
    perms = [np.argsort(-mask[b], kind="stable") for b in range(B)]
    m1max = max(int(mask[b].sum()) for b in range(B))
    NK = min(NT, max(1, -(-m1max // P)))
    KL = NK * P
    key = ("nc", NK)
    if key not in _CACHE:
        _CACHE[key] = _build(NK=NK)
    nc = _CACHE[key]
    bq = np.asarray(inputs["bq"], f32)
    bk = np.asarray(inputs["bk"], f32)
    bv = np.asarray(inputs["bv"], f32)
    bo = np.asarray(inputs["bo"], f32)
    ln_g = np.asarray(inputs["ln_g"], f32)
    ln_b = np.asarray(inputs["ln_b"], f32)

    shared = {
        "Wq": _chunk(Wq).astype(NPBF16),
        "Wk": _chunk(Wk).astype(NPBF16),
        "Wv": _chunk(Wv).astype(NPBF16),
        "Wo": _chunk(Wo).astype(NPBF16),
        "boe": (bv @ Wo + bo).reshape(1, C).astype(NPBF16),
        "lng": ln_g.reshape(1, C).astype(NPBF16),
        "lnb": ln_b.reshape(1, C).astype(NPBF16),
    }
    in_maps = []
    for core in range(8):
        b, half = core // 2, core % 2
        xt = np.ascontiguousarray(x[b].T)  # [C, T]
        pk = perms[b][:KL]
        mfp = mask[b][pk].astype(f32)     # permuted/truncated key mask
        vcol = np.zeros((P, NC), f32)
        vcol[:, :NK] = mfp.reshape(NK, P).T
        vecs = np.stack([bq.reshape(NC, P).T, bk.reshape(NC, P).T, vcol], axis=-1)
        m = dict(shared)
        m["xT"] = _chunk(np.ascontiguousarray(xt[:, pk])).astype(NPBF16)
        m["xTq"] = _chunk(xt[:, half * TQ : (half + 1) * TQ]).astype(NPBF16)
        m["vecs"] = np.ascontiguousarray(vecs, f32)
        m["maskv"] = np.ascontiguousarray(mfp.reshape(NK, P).T.astype(NPBF16))
        in_maps.append(m)

    trace = os.environ.get("KERNEL_TRACE", "0") == "1"
    if trace:
        _ensure_ntff_hook()
    LAST_RESULTS = run_bass_kernel_spmd(
        nc, in_maps, core_ids=list(range(8)), trace=trace
    )
    out = np.empty((B, T, C), f32)
    for core in range(8):
        b, half = core // 2, core % 2
        out[b, half * TQ : (half + 1) * TQ, :] = LAST_RESULTS.results[core][
            "y"
        ].astype(f32)
    return out


# revision 47
# speedup vs baseline: 1.0880x; 1.0576x over previous
"""Trainium2 Bass kernel for nn_MultiHeadAttention (B=4,T=1024,C=1024,H=16).

Sharding: 8 cores = 4 batches x 2 query-halves. Each core computes, for its
batch b and its 512 query rows:
  V projection (natural layout, mask folded in, + 64 replicated mask cols per
  head so the O matmul emits the softmax denominator pre-replicated across 64
  psum partitions), then per head-pair: Q^T/K^T projection chunks,
  S^T = K^T.T @ Q^T (row-packed head pairs, D=64 contraction), E^T =
  exp(0.125*S^T) on ACT (unmasked), O^T+den via one [V_h*m | m*64] matmul,
  normalize with reciprocal_approx_fast at partition base 0 (no DRAM bounce);
  finally Y = O^T.T @ Wo + (bv@Wo+bo) (bias via K=1 matmul) and LayerNorm with
  stats on PSUM, affine on ACT, bf16 output.
Host gathers the 8 [512,1024] bf16 outputs into [4,1024,1024] f32.
"""

import os
import sys

import numpy as np

for _p in ("/opt/trn_rl_repo", "/root/.axon_site/_ro/trn_rl_repo"):
    if os.path.isdir(_p) and _p not in sys.path:
        sys.path.append(_p)

import ml_dtypes  # noqa: E402
import concourse.bass as bass  # noqa: E402
import concourse.mybir as mybir  # noqa: E402
import concourse.tile as tile  # noqa: E402
from concourse import bacc  # noqa: E402
from concourse.bass_utils import run_bass_kernel_spmd  # noqa: E402

BF16 = mybir.dt.bfloat16
F32 = mybir.dt.float32
NPBF16 = ml_dtypes.bfloat16

B, T, C, H = 4, 1024, 1024, 16
D = C // H          # 64
P = 128             # partitions
NC = C // P         # 8 chunks of C
NT = T // P         # 8 chunks of T
TQ = T // 2         # 512 query rows per core
NQ = TQ // P        # 4 query chunks
NPAIR = H // 2      # 8 head pairs
EPS = 1e-5

_CACHE = {}
LAST_RESULTS = None


def _ensure_ntff_hook():
    """Register the axon NTFF profiling hook if the image's antenv lacks it."""
    try:
        import antenv.axon_hooks  # noqa: F401
        return
    except ImportError:
        pass
    try:
        import types

        import antenv
        from trn_agent_boot.trn_boot import _ntff_profile_via_ctypes

        mod = types.ModuleType("antenv.axon_hooks")
        state = {"hook": None}
        mod.set_axon_ntff_profile_hook = lambda h: state.__setitem__("hook", h)
        mod.get_axon_ntff_profile_hook = lambda: state["hook"]
        sys.modules["antenv.axon_hooks"] = mod
        antenv.axon_hooks = mod
        hook = _ntff_profile_via_ctypes("/opt/axon/libaxon_pjrt.so")
        if hook is not None:
            mod.set_axon_ntff_profile_hook(hook)
    except Exception:
        pass


def _emit(nc, tc, dr, NK, triv_gb, triv_boe):
    """Emit the per-core Tile program."""
    from contextlib import ExitStack

    with ExitStack() as ctx:
        consts = ctx.enter_context(tc.tile_pool(name="consts", bufs=1))

        KL = NK * P
        # VA[:, jc, h, :] = [V_h*m | m x64] for even h (O rows 0:64, den64
        # rows 64:128), [m x64 | V_h*m] for odd h (den64 low, O high). The
        # mask block gives the softmax denominator replicated across 64 psum
        # partitions for free in the O matmul.
        VA = consts.tile([P, NK, H, P], BF16)
        OT = consts.tile([P, NC, TQ], BF16)        # O^T (unnormed even/odd)
        Wo_sb = consts.tile([P, NC, C], BF16)
        lng_rep = lnb_rep = boe_sb = ones1 = None
        if not triv_gb:
            lng_rep = consts.tile([P, C], BF16)
            lnb_rep = consts.tile([P, C], BF16)
        if not triv_boe:
            boe_sb = consts.tile([1, C], BF16)     # bv@Wo + bo
            ones1 = consts.tile([1, P], BF16)
        vecs = consts.tile([P, NC, 3], F32)        # bq | bk | maskf
        maskv = consts.tile([P, NK], BF16)
        eps_t = consts.tile([P, 1], F32)

        nc.vector.memset(eps_t, EPS)
        if ones1 is not None:
            nc.vector.memset(ones1, 1.0)
        nc.gpsimd.dma_start(out=vecs[:], in_=dr["vecs"].ap()[:])
        nc.gpsimd.dma_start(out=maskv[:], in_=dr["maskv"].ap()[:])
        if not triv_boe:
            nc.gpsimd.dma_start(out=boe_sb[:], in_=dr["boe"].ap()[:])

        with (
            tc.tile_pool(name="pa", bufs=1) as pa,
            tc.tile_pool(name="pb", bufs=2) as pb,
            tc.tile_pool(name="psP", bufs=2, space="PSUM") as psP,
        ):
            xT = pa.tile([P, NC, KL], BF16)
            xTq = pa.tile([P, NC, TQ], BF16)
            Wq_sb = pa.tile([P, NC, C], BF16)
            Wk_sb = pa.tile([P, NC, C], BF16)
            Wv_sb = pa.tile([P, NC, C], BF16)
            # Input loads striped across the three DMA queues (sync/scalar/
            # gpsimd each stripe over all 16 DMA engines; aggregate ~350GB/s),
            # phased by first-use: xT+Wv lo | Wv hi | xTq+Wq | Wk | Wo.
            ENG = [nc.sync, nc.scalar, nc.gpsimd]

            def _ldsplit(dst, src):
                n = dst.shape[1]
                bnd = [0, (n + 2) // 3, n - (n + 2) // 3, n]
                bnd[2] += bnd[1]
                for e in range(3):
                    lo, hi = bnd[e], bnd[e + 1]
                    if hi > lo:
                        ENG[e].dma_start(
                            out=dst[:, lo:hi], in_=src[:, lo:hi]
                        )

            # V-proj inputs interleaved per kc chunk so mm(kc) can start as
            # soon as its own xT row + Wv column block land (~10.5us).
            for kc in range(NC):
                ENG[kc % 3].dma_start(
                    out=xT[:, kc, :], in_=dr["xT"].ap()[:, kc, :]
                )
                ENG[kc % 3].dma_start(
                    out=Wv_sb[:, kc, 0:TQ], in_=dr["Wv"].ap()[:, kc, 0:TQ]
                )
            _ldsplit(Wv_sb[:, :, TQ:], dr["Wv"].ap()[:, :, TQ:])
            _ldsplit(Wk_sb[:], dr["Wk"].ap()[:])
            _ldsplit(xTq[:], dr["xTq"].ap()[:])
            _ldsplit(Wq_sb[:], dr["Wq"].ap()[:])
            _ldsplit(Wo_sb[:], dr["Wo"].ap()[:])
            if not triv_gb:
                for name, rep in (("lng", lng_rep), ("lnb", lnb_rep)):
                    a = dr[name].ap()
                    bcast = bass.AP(
                        tensor=a.tensor, offset=a.offset, ap=[[0, P], [1, C]]
                    )
                    nc.gpsimd.dma_start(out=rep[:], in_=bcast)

            # ---- output projection + LayerNorm body (called per qc) ----
            def _qc_head(qc, psn):
                # mc 0..6 accumulation for both halves (no reads of the last
                # pair's OT chunk) — bridge work for the tensor queue.
                qs = slice(qc * P, (qc + 1) * P)
                for nn in range(2):
                    for mc in range(NC - 1):
                        nc.tensor.matmul(
                            psn[nn],
                            OT[:, mc, qs],
                            Wo_sb[:, mc, nn * TQ : (nn + 1) * TQ],
                            start=(mc == 0),
                            stop=False,
                        )

            def _qc_tail(qc, psn):
                qs = slice(qc * P, (qc + 1) * P)
                for nn in range(2):
                    nc.tensor.matmul(
                        psn[nn],
                        OT[:, NC - 1, qs],
                        Wo_sb[:, NC - 1, nn * TQ : (nn + 1) * TQ],
                        start=False, stop=triv_boe,
                    )
                    if not triv_boe:
                        nc.tensor.matmul(
                            psn[nn],
                            ones1[0:1, :],
                            boe_sb[0:1, nn * TQ : (nn + 1) * TQ],
                            start=False, stop=True,
                        )
                stats = pb.tile(
                    [P, 2, nc.vector.BN_STATS_DIM], F32, tag="stats"
                )
                mv = pb.tile([P, nc.vector.BN_AGGR_DIM], F32, tag="mv")
                nc.vector.bn_stats(out=stats[:, 0, :], in_=psn[0][:])
                nc.vector.bn_stats(out=stats[:, 1, :], in_=psn[1][:])
                nc.vector.bn_aggr(out=mv[:], in_=stats[:])
                rstd = pb.tile([P, 1], F32, tag="rstd")
                nmr = pb.tile([P, 1], F32, tag="nmr")
                nc.scalar.activation(
                    out=rstd[:], in_=mv[:, 1:2],
                    func=mybir.ActivationFunctionType.Sqrt,
                    bias=eps_t[:], scale=1.0,
                )
                nc.vector.reciprocal(out=rstd[:], in_=rstd[:])
                nc.vector.tensor_scalar(
                    nmr[:], mv[:, 0:1], rstd[:], -1.0,
                    mybir.AluOpType.mult, mybir.AluOpType.mult,
                )
                Yf = pb.tile([P, C], BF16, tag="yf")
                for nn in range(2):
                    sl = slice(nn * TQ, (nn + 1) * TQ)
                    nc.scalar.activation(
                        out=Yf[:, sl], in_=psn[nn][:],
                        func=mybir.ActivationFunctionType.Identity,
                        bias=nmr[:], scale=rstd[:],
                    )
                    if not triv_gb:
                        if not triv_gb:
                            nc.vector.tensor_tensor(
                                Yf[:, sl], Yf[:, sl], lng_rep[:, sl],
                                mybir.AluOpType.mult,
                            )
                            nc.vector.tensor_tensor(
                                Yf[:, sl], Yf[:, sl], lnb_rep[:, sl],
                                mybir.AluOpType.add,
                            )
                    (nc.sync if nn == 0 else nc.gpsimd).dma_start(
                        out=dr["y"].ap()[qs, sl], in_=Yf[:, sl]
                    )

            with (
                tc.tile_pool(name="psS", bufs=2, space="PSUM") as psS,
                tc.tile_pool(name="psO", bufs=2, space="PSUM") as psO,
            ):
                # ---- V projection: [keys, C] natural, masked, + mask block --
                for nn in range(2):
                    for tcn in range(NK):
                        ps = psP.tile([P, TQ], F32, tag="psp")
                        for kc in range(NC):
                            nc.tensor.matmul(
                                ps[:],
                                xT[:, kc, tcn * P : (tcn + 1) * P],
                                Wv_sb[:, kc, nn * TQ : (nn + 1) * TQ],
                                start=(kc == 0),
                                stop=(kc == NC - 1),
                            )
                        # V blocks land at +0 (even heads) / +64 (odd heads)
                        a = VA[:, tcn, :, :]
                        vdst = bass.AP(
                            tensor=a.tensor, offset=a.offset + nn * 8 * P,
                            ap=[a.ap[0], [2 * P, 4], [P + D, 2], [1, D]],
                        )
                        nc.vector.tensor_scalar_mul(
                            vdst,
                            ps[:].rearrange("p (a b d) -> p a b d", a=4, b=2),
                            vecs[:, tcn, 2:3],
                        )
                        if nn == 0:
                            # mask blocks: +64 for even heads, +0 for odd
                            mdst = bass.AP(
                                tensor=a.tensor, offset=a.offset + D,
                                ap=[a.ap[0], [2 * P, 8], [D, 2], [1, D]],
                            )
                            nc.vector.tensor_copy(
                                out=mdst,
                                in_=maskv[:, tcn, None].to_broadcast(
                                    (P, 8, 2, D)
                                ),
                            )

                # ---- per head-pair: QT/KT proj, S^T, exp, O^T, normalize,
                # software-pipelined so K/Q(c+1) matmuls cover exp(c) on ACT.
                def emit_kq(c):
                    QTc = pb.tile([P, TQ], BF16, tag="qtc")
                    KTc = pb.tile([P, KL], BF16, tag="ktc")
                    for ko in range(0, KL, TQ):
                        w = min(TQ, KL - ko)
                        ps = psP.tile([P, TQ], F32, tag="psp")
                        for kc in range(NC):
                            nc.tensor.matmul(
                                ps[:, :w],
                                Wk_sb[:, kc, c * P : (c + 1) * P],
                                xT[:, kc, ko : ko + w],
                                start=(kc == 0),
                                stop=(kc == NC - 1),
                            )
                        nc.vector.tensor_scalar_add(
                            KTc[:, ko : ko + w], ps[:, :w], vecs[:, c, 1:2]
                        )
                    ps = psP.tile([P, TQ], F32, tag="psp")
                    for kc in range(NC):
                        nc.tensor.matmul(
                            ps[:],
                            Wq_sb[:, kc, c * P : (c + 1) * P],
                            xTq[:, kc, :],
                            start=(kc == 0),
                            stop=(kc == NC - 1),
                        )
                    nc.vector.tensor_scalar_add(QTc[:], ps[:], vecs[:, c, 0:1])
                    return QTc, KTc

                def emit_s(c, QTc, KTc):
                    et0 = pb.tile([P, NK, TQ], BF16, tag="et0", bufs=2)
                    et1 = pb.tile([P, NK, TQ], BF16, tag="et1", bufs=2)
                    for jj in range((NK + 1) // 2):
                        nu = min(2, NK - 2 * jj)
                        s0 = psS.tile([P, 2 * TQ], F32, tag="s0", bufs=1)
                        s1 = psS.tile([P, 2 * TQ], F32, tag="s1", bufs=1)
                        for u in range(nu):
                            jc = 2 * jj + u
                            js = slice(jc * P, (jc + 1) * P)
                            nc.tensor.matmul(
                                s0[:, u * TQ : (u + 1) * TQ],
                                KTc[0:D, js],
                                QTc[0:D, :],
                                start=True, stop=True,
                                tile_position=(0, 0),
                            )
                            nc.tensor.matmul(
                                s1[:, u * TQ : (u + 1) * TQ],
                                KTc[D:P, js],
                                QTc[D:P, :],
                                start=True, stop=True,
                                tile_position=(D, 0),
                            )
                        nc.scalar.activation(
                            out=et0[:, 2 * jj : 2 * jj + nu, :],
                            in_=s0[:, : nu * TQ],
                            func=mybir.ActivationFunctionType.Exp, scale=0.125,
                        )
                        nc.scalar.activation(
                            out=et1[:, 2 * jj : 2 * jj + nu, :],
                            in_=s1[:, : nu * TQ],
                            func=mybir.ActivationFunctionType.Exp, scale=0.125,
                        )
                    return et0, et1

                def emit_o_den(c, et0, et1):
                    h0, h1 = 2 * c, 2 * c + 1
                    # O^T + den64: even head [O | den64], odd [den64 | O].
                    # The even head's den chain (ACT copy -> DMA shift ->
                    # recip) is emitted between the two O matmul groups so it
                    # overlaps the odd head's matmuls.
                    po0 = psO.tile([P, TQ], F32, tag="po0", bufs=1)
                    po1 = psO.tile([P, TQ], F32, tag="po1", bufs=1)
                    dt = pb.tile([P, 2, TQ], F32, tag="dt")
                    rr = pb.tile([P, 2, TQ], F32, tag="rr")
                    for jc in range(NK):
                        nc.tensor.matmul(
                            po0[:], VA[:, jc, h0, :], et0[:, jc, :],
                            start=(jc == 0), stop=(jc == NK - 1),
                        )
                    nc.scalar.activation(
                        out=dt[D:P, 0, :], in_=po0[D:P, :],
                        func=mybir.ActivationFunctionType.Identity,
                    )
                    nc.sync.dma_start(out=dt[0:D, 0, :], in_=dt[D:P, 0, :])
                    nc.vector.reciprocal_approx_fast(
                        out=rr[0:D, 0, :], in_=dt[0:D, 0, :]
                    )
                    for jc in range(NK):
                        nc.tensor.matmul(
                            po1[:], VA[:, jc, h1, :], et1[:, jc, :],
                            start=(jc == 0), stop=(jc == NK - 1),
                        )
                    nc.vector.reciprocal_approx_fast(
                        out=rr[0:D, 1, :], in_=po1[0:D, :]
                    )
                    nc.sync.dma_start(out=rr[D:P, 1, :], in_=rr[0:D, 1, :])
                    nc.vector.tensor_tensor(
                        OT[0:D, c, :], po0[0:D, :], rr[0:D, 0, :],
                        mybir.AluOpType.mult,
                    )
                    nc.vector.tensor_tensor(
                        OT[D:P, c, :], po1[D:P, :], rr[D:P, 1, :],
                        mybir.AluOpType.mult,
                    )

                ets = emit_s(0, *emit_kq(0))
                for c in range(NPAIR):
                    nxt = emit_kq(c + 1) if c + 1 < NPAIR else None
                    emit_o_den(c, *ets)
                    if nxt is not None:
                        ets = emit_s(c + 1, *nxt)

                # qc 0/1 emitted inside the psS/psO scope from psP so their
                # matmuls bridge the pool-close barrier (which waits on the
                # last pair's den chain); the rest pipeline deeply from psL.
                # Bridge the pool-close barrier: four mc0..6 chains emitted
                # before any mc7 tail (the in-order tensor queue would
                # otherwise block on the last pair's OT chunk). qc1's psums
                # reuse the psS slots, free right after pair 7's exp.
                a0 = psP.tile([P, TQ], F32, tag="psp")
                a1 = psP.tile([P, TQ], F32, tag="psp")
                b0 = psO.tile([P, TQ], F32, tag="po0", bufs=1)
                b1 = psO.tile([P, TQ], F32, tag="po1", bufs=1)
                qc0 = [a0[:], a1[:]]
                qc1 = [b0[:], b1[:]]
                _qc_head(0, qc0)
                _qc_head(1, qc1)
                _qc_tail(0, qc0)
                _qc_tail(1, qc1)
            with tc.tile_pool(name="psL", bufs=4, space="PSUM") as psL:
                for qc in range(2, NQ):
                    p0 = psL.tile([P, TQ], F32, tag="psl")
                    p1 = psL.tile([P, TQ], F32, tag="psl")
                    psn = [p0[:], p1[:]]
                    if qc < NQ - 1:
                        _qc_head(qc, psn)
                        _qc_tail(qc, psn)
                        continue
                    # last qc: finish half 0 completely first so its
                    # bn_stats overlaps half 1's matmuls, shortening the
                    # exposed end-of-kernel LN chain.
                    qs = slice(qc * P, (qc + 1) * P)
                    stats = pb.tile(
                        [P, 2, nc.vector.BN_STATS_DIM], F32, tag="stats"
                    )
                    for nn in range(2):
                        for mc in range(NC):
                            nc.tensor.matmul(
                                psn[nn],
                                OT[:, mc, qs],
                                Wo_sb[:, mc, nn * TQ : (nn + 1) * TQ],
                                start=(mc == 0),
                                stop=(triv_boe and mc == NC - 1),
                            )
                        if not triv_boe:
                            nc.tensor.matmul(
                                psn[nn],
                                ones1[0:1, :],
                                boe_sb[0:1, nn * TQ : (nn + 1) * TQ],
                                start=False, stop=True,
                            )
                        nc.vector.bn_stats(out=stats[:, nn, :], in_=psn[nn])
                    mv = pb.tile([P, nc.vector.BN_AGGR_DIM], F32, tag="mv")
                    nc.vector.bn_aggr(out=mv[:], in_=stats[:])
                    rstd = pb.tile([P, 1], F32, tag="rstd")
                    nmr = pb.tile([P, 1], F32, tag="nmr")
                    nc.scalar.activation(
                        out=rstd[:], in_=mv[:, 1:2],
                        func=mybir.ActivationFunctionType.Sqrt,
                        bias=eps_t[:], scale=1.0,
                    )
                    nc.vector.reciprocal(out=rstd[:], in_=rstd[:])
                    nc.vector.tensor_scalar(
                        nmr[:], mv[:, 0:1], rstd[:], -1.0,
                        mybir.AluOpType.mult, mybir.AluOpType.mult,
                    )
                    Yf = pb.tile([P, C], BF16, tag="yf")
                    for nn in range(2):
                        sl = slice(nn * TQ, (nn + 1) * TQ)
                        nc.scalar.activation(
                            out=Yf[:, sl], in_=psn[nn],
                            func=mybir.ActivationFunctionType.Identity,
                            bias=nmr[:], scale=rstd[:],
                        )
                        if not triv_gb:
                            nc.vector.tensor_tensor(
                                Yf[:, sl], Yf[:, sl], lng_rep[:, sl],
                                mybir.AluOpType.mult,
                            )
                            nc.vector.tensor_tensor(
                                Yf[:, sl], Yf[:, sl], lnb_rep[:, sl],
                                mybir.AluOpType.add,
                            )
                        (nc.sync if nn == 0 else nc.gpsimd).dma_start(
                            out=dr["y"].ap()[qs, sl], in_=Yf[:, sl]
                        )


def _build(NK=NT, triv_gb=False, triv_boe=False):
    nc = bacc.Bacc("TRN2", target_bir_lowering=False, debug=False, num_devices=8)
    dr = {}
    dr["xT"] = nc.dram_tensor("xT", [P, NC, NK * P], BF16, kind="ExternalInput")
    dr["xTq"] = nc.dram_tensor("xTq", [P, NC, TQ], BF16, kind="ExternalInput")
    for w in ("Wq", "Wk", "Wv", "Wo"):
        dr[w] = nc.dram_tensor(w, [P, NC, C], BF16, kind="ExternalInput")
    dr["vecs"] = nc.dram_tensor("vecs", [P, NC, 3], F32, kind="ExternalInput")
    dr["maskv"] = nc.dram_tensor("maskv", [P, NK], BF16, kind="ExternalInput")
    dr["boe"] = nc.dram_tensor("boe", [1, C], BF16, kind="ExternalInput")
    for v in ("lng", "lnb"):
        dr[v] = nc.dram_tensor(v, [1, C], BF16, kind="ExternalInput")
    dr["y"] = nc.dram_tensor("y", [TQ, C], BF16, kind="ExternalOutput")
    with tile.TileContext(nc) as tc:
        _emit(nc, tc, dr, NK, triv_gb, triv_boe)
    nc.compile()
    return nc


def _chunk(a):
    """[C, N] -> [128, C//128, N] with [p, c, n] = a[128c+p, n]."""
    return np.ascontiguousarray(
        a.reshape(NC, P, -1).transpose(1, 0, 2)
    )


def kernel(**inputs):
    global LAST_RESULTS
    f32 = np.float32
    Wq = np.asarray(inputs["Wq"], f32)
    Wk = np.asarray(inputs["Wk"], f32)
    Wv = np.asarray(inputs["Wv"], f32)
    Wo = np.asarray(inputs["Wo"], f32)
    x = np.asarray(inputs["x"], f32)
    mask = np.asarray(inputs["attn_mask"]).reshape(B, T)
    # sort keys so unmasked come first; masked tail chunks are dropped
    perms = [np.argsort(-mask[b], kind="stable") for b in range(B)]
    m1max = max(int(mask[b].sum()) for b in range(B))
    NK = min(NT, max(1, -(-m1max // P)))
    KL = NK * P
    bq = np.asarray(inputs["bq"], f32)
    bk = np.asarray(inputs["bk"], f32)
    bv = np.asarray(inputs["bv"], f32)
    bo = np.asarray(inputs["bo"], f32)
    ln_g = np.asarray(inputs["ln_g"], f32)
    ln_b = np.asarray(inputs["ln_b"], f32)
    boe_v = (bv @ Wo + bo).astype(f32)
    triv_gb = bool(np.all(ln_g == 1.0) and np.all(ln_b == 0.0))
    triv_boe = bool(np.all(boe_v == 0.0))
    key = ("nc", NK, triv_gb, triv_boe)
    if key not in _CACHE:
        _CACHE[key] = _build(NK=NK, triv_gb=triv_gb, triv_boe=triv_boe)
    nc = _CACHE[key]

    shared = {
        "Wq": _chunk(Wq).astype(NPBF16),
        "Wk": _chunk(Wk).astype(NPBF16),
        "Wv": _chunk(Wv).astype(NPBF16),
        "Wo": _chunk(Wo).astype(NPBF16),
        "boe": (bv @ Wo + bo).reshape(1, C).astype(NPBF16),
        "lng": ln_g.reshape(1, C).astype(NPBF16),
        "lnb": ln_b.reshape(1, C).astype(NPBF16),
    }
    in_maps = []
    for core in range(8):
        b, half = core // 2, core % 2
        xt = np.ascontiguousarray(x[b].T)  # [C, T]
        pk = perms[b][:KL]
        mfp = mask[b][pk].astype(f32)     # permuted/truncated key mask
        vcol = np.zeros((P, NC), f32)
        vcol[:, :NK] = mfp.reshape(NK, P).T
        vecs = np.stack([bq.reshape(NC, P).T, bk.reshape(NC, P).T, vcol], axis=-1)
        m = dict(shared)
        m["xT"] = _chunk(np.ascontiguousarray(xt[:, pk])).astype(NPBF16)
        m["xTq"] = _chunk(xt[:, half * TQ : (half + 1) * TQ]).astype(NPBF16)
        m["vecs"] = np.ascontiguousarray(vecs, f32)
        m["maskv"] = np.ascontiguousarray(mfp.reshape(NK, P).T.astype(NPBF16))
        in_maps.append(m)

    trace = os.environ.get("KERNEL_TRACE", "0") == "1"
    if trace:
        _ensure_ntff_hook()
    LAST_RESULTS = run_bass_kernel_spmd(
        nc, in_maps, core_ids=list(range(8)), trace=trace
    )
    out = np.empty((B, T, C), f32)
    for core in range(8):
        b, half = core // 2, core % 2
        out[b, half * TQ : (half + 1) * TQ, :] = LAST_RESULTS.results[core][
            "y"
        ].astype(f32)
    return out


# revision 48
# speedup vs baseline: 1.0912x; 1.0030x over previous
"""Trainium2 Bass kernel for nn_MultiHeadAttention (B=4,T=1024,C=1024,H=16).

Sharding: 8 cores = 4 batches x 2 query-halves. Each core computes, for its
batch b and its 512 query rows:
  V projection (natural layout, mask folded in, + 64 replicated mask cols per
  head so the O matmul emits the softmax denominator pre-replicated across 64
  psum partitions), then per head-pair: Q^T/K^T projection chunks,
  S^T = K^T.T @ Q^T (row-packed head pairs, D=64 contraction), E^T =
  exp(0.125*S^T) on ACT (unmasked), O^T+den via one [V_h*m | m*64] matmul,
  normalize with reciprocal_approx_fast at partition base 0 (no DRAM bounce);
  finally Y = O^T.T @ Wo + (bv@Wo+bo) (bias via K=1 matmul) and LayerNorm with
  stats on PSUM, affine on ACT, bf16 output.
Host gathers the 8 [512,1024] bf16 outputs into [4,1024,1024] f32.
"""

import os
import sys

import numpy as np

for _p in ("/opt/trn_rl_repo", "/root/.axon_site/_ro/trn_rl_repo"):
    if os.path.isdir(_p) and _p not in sys.path:
        sys.path.append(_p)

import ml_dtypes  # noqa: E402
import concourse.bass as bass  # noqa: E402
import concourse.mybir as mybir  # noqa: E402
import concourse.tile as tile  # noqa: E402
from concourse import bacc  # noqa: E402
from concourse.bass_utils import run_bass_kernel_spmd  # noqa: E402

BF16 = mybir.dt.bfloat16
F32 = mybir.dt.float32
NPBF16 = ml_dtypes.bfloat16

B, T, C, H = 4, 1024, 1024, 16
D = C // H          # 64
P = 128             # partitions
NC = C // P         # 8 chunks of C
NT = T // P         # 8 chunks of T
TQ = T // 2         # 512 query rows per core
NQ = TQ // P        # 4 query chunks
NPAIR = H // 2      # 8 head pairs
EPS = 1e-5

_CACHE = {}
LAST_RESULTS = None


def _ensure_ntff_hook():
    """Register the axon NTFF profiling hook if the image's antenv lacks it."""
    try:
        import antenv.axon_hooks  # noqa: F401
        return
    except ImportError:
        pass
    try:
        import types

        import antenv
        from trn_agent_boot.trn_boot import _ntff_profile_via_ctypes

        mod = types.ModuleType("antenv.axon_hooks")
        state = {"hook": None}
        mod.set_axon_ntff_profile_hook = lambda h: state.__setitem__("hook", h)
        mod.get_axon_ntff_profile_hook = lambda: state["hook"]
        sys.modules["antenv.axon_hooks"] = mod
        antenv.axon_hooks = mod
        hook = _ntff_profile_via_ctypes("/opt/axon/libaxon_pjrt.so")
        if hook is not None:
            mod.set_axon_ntff_profile_hook(hook)
    except Exception:
        pass


def _emit(nc, tc, dr, NK, triv_gb, triv_boe):
    """Emit the per-core Tile program."""
    from contextlib import ExitStack

    with ExitStack() as ctx:
        consts = ctx.enter_context(tc.tile_pool(name="consts", bufs=1))

        KL = NK * P
        # VA[:, jc, h, :] = [V_h*m | m x64] for even h (O rows 0:64, den64
        # rows 64:128), [m x64 | V_h*m] for odd h (den64 low, O high). The
        # mask block gives the softmax denominator replicated across 64 psum
        # partitions for free in the O matmul.
        VA = consts.tile([P, NK, H, P], BF16)
        OT = consts.tile([P, NC, TQ], BF16)        # O^T (unnormed even/odd)
        Wo_sb = consts.tile([P, NC, C], BF16)
        lng_rep = lnb_rep = boe_sb = ones1 = None
        if not triv_gb:
            lng_rep = consts.tile([P, C], BF16)
            lnb_rep = consts.tile([P, C], BF16)
        if not triv_boe:
            boe_sb = consts.tile([1, C], BF16)     # bv@Wo + bo
            ones1 = consts.tile([1, P], BF16)
        vecs = consts.tile([P, NC, 3], F32)        # bq | bk | maskf
        maskv = consts.tile([P, NK], BF16)
        eps_t = consts.tile([P, 1], F32)

        nc.vector.memset(eps_t, EPS)
        if ones1 is not None:
            nc.vector.memset(ones1, 1.0)
        nc.gpsimd.dma_start(out=vecs[:], in_=dr["vecs"].ap()[:])
        nc.gpsimd.dma_start(out=maskv[:], in_=dr["maskv"].ap()[:])
        if not triv_boe:
            nc.gpsimd.dma_start(out=boe_sb[:], in_=dr["boe"].ap()[:])

        with (
            tc.tile_pool(name="pa", bufs=1) as pa,
            tc.tile_pool(name="pb", bufs=2) as pb,
            tc.tile_pool(name="psP", bufs=2, space="PSUM") as psP,
        ):
            xT = pa.tile([P, NC, KL], BF16)
            xTq = pa.tile([P, NC, TQ], BF16)
            Wq_sb = pa.tile([P, NC, C], BF16)
            Wk_sb = pa.tile([P, NC, C], BF16)
            Wv_sb = pa.tile([P, NC, C], BF16)
            # Input loads striped across the three DMA queues (sync/scalar/
            # gpsimd each stripe over all 16 DMA engines; aggregate ~350GB/s),
            # phased by first-use: xT+Wv lo | Wv hi | xTq+Wq | Wk | Wo.
            ENG = [nc.sync, nc.scalar, nc.gpsimd]

            def _ldsplit(dst, src):
                n = dst.shape[1]
                bnd = [0, (n + 2) // 3, n - (n + 2) // 3, n]
                bnd[2] += bnd[1]
                for e in range(3):
                    lo, hi = bnd[e], bnd[e + 1]
                    if hi > lo:
                        ENG[e].dma_start(
                            out=dst[:, lo:hi], in_=src[:, lo:hi]
                        )

            # V-proj inputs interleaved per kc chunk so mm(kc) can start as
            # soon as its own xT row + Wv column block land (~10.5us).
            for kc in range(NC):
                ENG[kc % 3].dma_start(
                    out=xT[:, kc, :], in_=dr["xT"].ap()[:, kc, :]
                )
                ENG[kc % 3].dma_start(
                    out=Wv_sb[:, kc, 0:TQ], in_=dr["Wv"].ap()[:, kc, 0:TQ]
                )
            _ldsplit(Wv_sb[:, :, TQ:], dr["Wv"].ap()[:, :, TQ:])
            _ldsplit(Wk_sb[:], dr["Wk"].ap()[:])
            _ldsplit(xTq[:], dr["xTq"].ap()[:])
            _ldsplit(Wq_sb[:], dr["Wq"].ap()[:])
            _ldsplit(Wo_sb[:], dr["Wo"].ap()[:])
            if not triv_gb:
                for name, rep in (("lng", lng_rep), ("lnb", lnb_rep)):
                    a = dr[name].ap()
                    bcast = bass.AP(
                        tensor=a.tensor, offset=a.offset, ap=[[0, P], [1, C]]
                    )
                    nc.gpsimd.dma_start(out=rep[:], in_=bcast)

            # ---- output projection + LayerNorm body (called per qc) ----
            def _qc_head(qc, psn):
                # mc 0..6 accumulation for both halves (no reads of the last
                # pair's OT chunk) — bridge work for the tensor queue.
                qs = slice(qc * P, (qc + 1) * P)
                for nn in range(2):
                    for mc in range(NC - 1):
                        nc.tensor.matmul(
                            psn[nn],
                            OT[:, mc, qs],
                            Wo_sb[:, mc, nn * TQ : (nn + 1) * TQ],
                            start=(mc == 0),
                            stop=False,
                        )

            def _qc_tail(qc, psn):
                qs = slice(qc * P, (qc + 1) * P)
                for nn in range(2):
                    nc.tensor.matmul(
                        psn[nn],
                        OT[:, NC - 1, qs],
                        Wo_sb[:, NC - 1, nn * TQ : (nn + 1) * TQ],
                        start=False, stop=triv_boe,
                    )
                    if not triv_boe:
                        nc.tensor.matmul(
                            psn[nn],
                            ones1[0:1, :],
                            boe_sb[0:1, nn * TQ : (nn + 1) * TQ],
                            start=False, stop=True,
                        )
                stats = pb.tile(
                    [P, 2, nc.vector.BN_STATS_DIM], F32, tag="stats"
                )
                mv = pb.tile([P, nc.vector.BN_AGGR_DIM], F32, tag="mv")
                nc.vector.bn_stats(out=stats[:, 0, :], in_=psn[0][:])
                nc.vector.bn_stats(out=stats[:, 1, :], in_=psn[1][:])
                nc.vector.bn_aggr(out=mv[:], in_=stats[:])
                rstd = pb.tile([P, 1], F32, tag="rstd")
                nmr = pb.tile([P, 1], F32, tag="nmr")
                nc.scalar.activation(
                    out=rstd[:], in_=mv[:, 1:2],
                    func=mybir.ActivationFunctionType.Sqrt,
                    bias=eps_t[:], scale=1.0,
                )
                nc.vector.reciprocal(out=rstd[:], in_=rstd[:])
                nc.vector.tensor_scalar(
                    nmr[:], mv[:, 0:1], rstd[:], -1.0,
                    mybir.AluOpType.mult, mybir.AluOpType.mult,
                )
                Yf = pb.tile([P, C], BF16, tag="yf")
                for nn in range(2):
                    sl = slice(nn * TQ, (nn + 1) * TQ)
                    # split the affine across engines so the two halves of
                    # each qc run concurrently in the tail
                    if nn == 0:
                        nc.scalar.activation(
                            out=Yf[:, sl], in_=psn[nn][:],
                            func=mybir.ActivationFunctionType.Identity,
                            bias=nmr[:], scale=rstd[:],
                        )
                    else:
                        nc.vector.tensor_scalar(
                            Yf[:, sl], psn[nn][:], rstd[:], nmr[:],
                            mybir.AluOpType.mult, mybir.AluOpType.add,
                        )
                    if not triv_gb:
                        if not triv_gb:
                            nc.vector.tensor_tensor(
                                Yf[:, sl], Yf[:, sl], lng_rep[:, sl],
                                mybir.AluOpType.mult,
                            )
                            nc.vector.tensor_tensor(
                                Yf[:, sl], Yf[:, sl], lnb_rep[:, sl],
                                mybir.AluOpType.add,
                            )
                    (nc.sync if nn == 0 else nc.gpsimd).dma_start(
                        out=dr["y"].ap()[qs, sl], in_=Yf[:, sl]
                    )

            with (
                tc.tile_pool(name="psS", bufs=2, space="PSUM") as psS,
                tc.tile_pool(name="psO", bufs=2, space="PSUM") as psO,
            ):
                # ---- V projection: [keys, C] natural, masked, + mask block --
                for nn in range(2):
                    for tcn in range(NK):
                        ps = psP.tile([P, TQ], F32, tag="psp")
                        for kc in range(NC):
                            nc.tensor.matmul(
                                ps[:],
                                xT[:, kc, tcn * P : (tcn + 1) * P],
                                Wv_sb[:, kc, nn * TQ : (nn + 1) * TQ],
                                start=(kc == 0),
                                stop=(kc == NC - 1),
                            )
                        # V blocks land at +0 (even heads) / +64 (odd heads)
                        a = VA[:, tcn, :, :]
                        vdst = bass.AP(
                            tensor=a.tensor, offset=a.offset + nn * 8 * P,
                            ap=[a.ap[0], [2 * P, 4], [P + D, 2], [1, D]],
                        )
                        nc.vector.tensor_scalar_mul(
                            vdst,
                            ps[:].rearrange("p (a b d) -> p a b d", a=4, b=2),
                            vecs[:, tcn, 2:3],
                        )
                        if nn == 0:
                            # mask blocks: +64 for even heads, +0 for odd
                            mdst = bass.AP(
                                tensor=a.tensor, offset=a.offset + D,
                                ap=[a.ap[0], [2 * P, 8], [D, 2], [1, D]],
                            )
                            nc.vector.tensor_copy(
                                out=mdst,
                                in_=maskv[:, tcn, None].to_broadcast(
                                    (P, 8, 2, D)
                                ),
                            )

                # ---- per head-pair: QT/KT proj, S^T, exp, O^T, normalize,
                # software-pipelined so K/Q(c+1) matmuls cover exp(c) on ACT.
                def emit_kq(c):
                    QTc = pb.tile([P, TQ], BF16, tag="qtc")
                    KTc = pb.tile([P, KL], BF16, tag="ktc")
                    for ko in range(0, KL, TQ):
                        w = min(TQ, KL - ko)
                        ps = psP.tile([P, TQ], F32, tag="psp")
                        for kc in range(NC):
                            nc.tensor.matmul(
                                ps[:, :w],
                                Wk_sb[:, kc, c * P : (c + 1) * P],
                                xT[:, kc, ko : ko + w],
                                start=(kc == 0),
                                stop=(kc == NC - 1),
                            )
                        nc.vector.tensor_scalar_add(
                            KTc[:, ko : ko + w], ps[:, :w], vecs[:, c, 1:2]
                        )
                    ps = psP.tile([P, TQ], F32, tag="psp")
                    for kc in range(NC):
                        nc.tensor.matmul(
                            ps[:],
                            Wq_sb[:, kc, c * P : (c + 1) * P],
                            xTq[:, kc, :],
                            start=(kc == 0),
                            stop=(kc == NC - 1),
                        )
                    nc.vector.tensor_scalar_add(QTc[:], ps[:], vecs[:, c, 0:1])
                    return QTc, KTc

                def emit_s(c, QTc, KTc):
                    et0 = pb.tile([P, NK, TQ], BF16, tag="et0", bufs=2)
                    et1 = pb.tile([P, NK, TQ], BF16, tag="et1", bufs=2)
                    for jj in range((NK + 1) // 2):
                        nu = min(2, NK - 2 * jj)
                        s0 = psS.tile([P, 2 * TQ], F32, tag="s0", bufs=1)
                        s1 = psS.tile([P, 2 * TQ], F32, tag="s1", bufs=1)
                        for u in range(nu):
                            jc = 2 * jj + u
                            js = slice(jc * P, (jc + 1) * P)
                            nc.tensor.matmul(
                                s0[:, u * TQ : (u + 1) * TQ],
                                KTc[0:D, js],
                                QTc[0:D, :],
                                start=True, stop=True,
                                tile_position=(0, 0),
                            )
                            nc.tensor.matmul(
                                s1[:, u * TQ : (u + 1) * TQ],
                                KTc[D:P, js],
                                QTc[D:P, :],
                                start=True, stop=True,
                                tile_position=(D, 0),
                            )
                        nc.scalar.activation(
                            out=et0[:, 2 * jj : 2 * jj + nu, :],
                            in_=s0[:, : nu * TQ],
                            func=mybir.ActivationFunctionType.Exp, scale=0.125,
                        )
                        nc.scalar.activation(
                            out=et1[:, 2 * jj : 2 * jj + nu, :],
                            in_=s1[:, : nu * TQ],
                            func=mybir.ActivationFunctionType.Exp, scale=0.125,
                        )
                    return et0, et1

                def emit_o_den(c, et0, et1):
                    h0, h1 = 2 * c, 2 * c + 1
                    # O^T + den64: even head [O | den64], odd [den64 | O].
                    # The even head's den chain (ACT copy -> DMA shift ->
                    # recip) is emitted between the two O matmul groups so it
                    # overlaps the odd head's matmuls.
                    po0 = psO.tile([P, TQ], F32, tag="po0", bufs=1)
                    po1 = psO.tile([P, TQ], F32, tag="po1", bufs=1)
                    dt = pb.tile([P, 2, TQ], F32, tag="dt")
                    rr = pb.tile([P, 2, TQ], F32, tag="rr")
                    for jc in range(NK):
                        nc.tensor.matmul(
                            po0[:], VA[:, jc, h0, :], et0[:, jc, :],
                            start=(jc == 0), stop=(jc == NK - 1),
                        )
                    nc.scalar.activation(
                        out=dt[D:P, 0, :], in_=po0[D:P, :],
                        func=mybir.ActivationFunctionType.Identity,
                    )
                    nc.sync.dma_start(out=dt[0:D, 0, :], in_=dt[D:P, 0, :])
                    nc.vector.reciprocal_approx_fast(
                        out=rr[0:D, 0, :], in_=dt[0:D, 0, :]
                    )
                    for jc in range(NK):
                        nc.tensor.matmul(
                            po1[:], VA[:, jc, h1, :], et1[:, jc, :],
                            start=(jc == 0), stop=(jc == NK - 1),
                        )
                    nc.vector.reciprocal_approx_fast(
                        out=rr[0:D, 1, :], in_=po1[0:D, :]
                    )
                    nc.sync.dma_start(out=rr[D:P, 1, :], in_=rr[0:D, 1, :])
                    nc.vector.tensor_tensor(
                        OT[0:D, c, :], po0[0:D, :], rr[0:D, 0, :],
                        mybir.AluOpType.mult,
                    )
                    nc.vector.tensor_tensor(
                        OT[D:P, c, :], po1[D:P, :], rr[D:P, 1, :],
                        mybir.AluOpType.mult,
                    )

                ets = emit_s(0, *emit_kq(0))
                for c in range(NPAIR):
                    nxt = emit_kq(c + 1) if c + 1 < NPAIR else None
                    emit_o_den(c, *ets)
                    if nxt is not None:
                        ets = emit_s(c + 1, *nxt)

                # qc 0/1 emitted inside the psS/psO scope from psP so their
                # matmuls bridge the pool-close barrier (which waits on the
                # last pair's den chain); the rest pipeline deeply from psL.
                # Bridge the pool-close barrier: four mc0..6 chains emitted
                # before any mc7 tail (the in-order tensor queue would
                # otherwise block on the last pair's OT chunk). qc1's psums
                # reuse the psS slots, free right after pair 7's exp.
                a0 = psP.tile([P, TQ], F32, tag="psp")
                a1 = psP.tile([P, TQ], F32, tag="psp")
                b0 = psO.tile([P, TQ], F32, tag="po0", bufs=1)
                b1 = psO.tile([P, TQ], F32, tag="po1", bufs=1)
                qc0 = [a0[:], a1[:]]
                qc1 = [b0[:], b1[:]]
                _qc_head(0, qc0)
                _qc_head(1, qc1)
                _qc_tail(0, qc0)
                _qc_tail(1, qc1)
            with tc.tile_pool(name="psL", bufs=4, space="PSUM") as psL:
                for qc in range(2, NQ):
                    p0 = psL.tile([P, TQ], F32, tag="psl")
                    p1 = psL.tile([P, TQ], F32, tag="psl")
                    psn = [p0[:], p1[:]]
                    if qc < NQ - 1:
                        _qc_head(qc, psn)
                        _qc_tail(qc, psn)
                        continue
                    # last qc: finish half 0 completely first so its
                    # bn_stats overlaps half 1's matmuls, shortening the
                    # exposed end-of-kernel LN chain.
                    qs = slice(qc * P, (qc + 1) * P)
                    stats = pb.tile(
                        [P, 2, nc.vector.BN_STATS_DIM], F32, tag="stats"
                    )
                    for nn in range(2):
                        for mc in range(NC):
                            nc.tensor.matmul(
                                psn[nn],
                                OT[:, mc, qs],
                                Wo_sb[:, mc, nn * TQ : (nn + 1) * TQ],
                                start=(mc == 0),
                                stop=(triv_boe and mc == NC - 1),
                            )
                        if not triv_boe:
                            nc.tensor.matmul(
                                psn[nn],
                                ones1[0:1, :],
                                boe_sb[0:1, nn * TQ : (nn + 1) * TQ],
                                start=False, stop=True,
                            )
                        nc.vector.bn_stats(out=stats[:, nn, :], in_=psn[nn])
                    mv = pb.tile([P, nc.vector.BN_AGGR_DIM], F32, tag="mv")
                    nc.vector.bn_aggr(out=mv[:], in_=stats[:])
                    rstd = pb.tile([P, 1], F32, tag="rstd")
                    nmr = pb.tile([P, 1], F32, tag="nmr")
                    nc.scalar.activation(
                        out=rstd[:], in_=mv[:, 1:2],
                        func=mybir.ActivationFunctionType.Sqrt,
                        bias=eps_t[:], scale=1.0,
                    )
                    nc.vector.reciprocal(out=rstd[:], in_=rstd[:])
                    nc.vector.tensor_scalar(
                        nmr[:], mv[:, 0:1], rstd[:], -1.0,
                        mybir.AluOpType.mult, mybir.AluOpType.mult,
                    )
                    Yf = pb.tile([P, C], BF16, tag="yf")
                    for nn in range(2):
                        sl = slice(nn * TQ, (nn + 1) * TQ)
                        if nn == 0:
                            nc.scalar.activation(
                                out=Yf[:, sl], in_=psn[nn],
                                func=mybir.ActivationFunctionType.Identity,
                                bias=nmr[:], scale=rstd[:],
                            )
                        else:
                            nc.vector.tensor_scalar(
                                Yf[:, sl], psn[nn], rstd[:], nmr[:],
                                mybir.AluOpType.mult, mybir.AluOpType.add,
                            )
                        if not triv_gb:
                            nc.vector.tensor_tensor(
                                Yf[:, sl], Yf[:, sl], lng_rep[:, sl],
                                mybir.AluOpType.mult,
                            )
                            nc.vector.tensor_tensor(
                                Yf[:, sl], Yf[:, sl], lnb_rep[:, sl],
                                mybir.AluOpType.add,
                            )
                        (nc.sync if nn == 0 else nc.gpsimd).dma_start(
                            out=dr["y"].ap()[qs, sl], in_=Yf[:, sl]
                        )


def _build(NK=NT, triv_gb=False, triv_boe=False):
    nc = bacc.Bacc("TRN2", target_bir_lowering=False, debug=False, num_devices=8)
    dr = {}
    dr["xT"] = nc.dram_tensor("xT", [P, NC, NK * P], BF16, kind="ExternalInput")
    dr["xTq"] = nc.dram_tensor("xTq", [P, NC, TQ], BF16, kind="ExternalInput")
    for w in ("Wq", "Wk", "Wv", "Wo"):
        dr[w] = nc.dram_tensor(w, [P, NC, C], BF16, kind="ExternalInput")
    dr["vecs"] = nc.dram_tensor("vecs", [P, NC, 3], F32, kind="ExternalInput")
    dr["maskv"] = nc.dram_tensor("maskv", [P, NK], BF16, kind="ExternalInput")
    dr["boe"] = nc.dram_tensor("boe", [1, C], BF16, kind="ExternalInput")
    for v in ("lng", "lnb"):
        dr[v] = nc.dram_tensor(v, [1, C], BF16, kind="ExternalInput")
    dr["y"] = nc.dram_tensor("y", [TQ, C], BF16, kind="ExternalOutput")
    with tile.TileContext(nc) as tc:
        _emit(nc, tc, dr, NK, triv_gb, triv_boe)
    nc.compile()
    return nc


def _chunk(a):
    """[C, N] -> [128, C//128, N] with [p, c, n] = a[128c+p, n]."""
    return np.ascontiguousarray(
        a.reshape(NC, P, -1).transpose(1, 0, 2)
    )


def kernel(**inputs):
    global LAST_RESULTS
    f32 = np.float32
    Wq = np.asarray(inputs["Wq"], f32)
    Wk = np.asarray(inputs["Wk"], f32)
    Wv = np.asarray(inputs["Wv"], f32)
    Wo = np.asarray(inputs["Wo"], f32)
    x = np.asarray(inputs["x"], f32)
    mask = np.asarray(inputs["attn_mask"]).reshape(B, T)
    # sort keys so unmasked come first; masked tail chunks are dropped
    perms = [np.argsort(-mask[b], kind="stable") for b in range(B)]
    m1max = max(int(mask[b].sum()) for b in range(B))
    NK = min(NT, max(1, -(-m1max // P)))
    KL = NK * P
    bq = np.asarray(inputs["bq"], f32)
    bk = np.asarray(inputs["bk"], f32)
    bv = np.asarray(inputs["bv"], f32)
    bo = np.asarray(inputs["bo"], f32)
    ln_g = np.asarray(inputs["ln_g"], f32)
    ln_b = np.asarray(inputs["ln_b"], f32)
    boe_v = (bv @ Wo + bo).astype(f32)
    triv_gb = bool(np.all(ln_g == 1.0) and np.all(ln_b == 0.0))
    triv_boe = bool(np.all(boe_v == 0.0))
    key = ("nc", NK, triv_gb, triv_boe)
    if key not in _CACHE:
        _CACHE[key] = _build(NK=NK, triv_gb=triv_gb, triv_boe=triv_boe)
    nc = _CACHE[key]

    shared = {
        "Wq": _chunk(Wq).astype(NPBF16),
        "Wk": _chunk(Wk).astype(NPBF16),
        "Wv": _chunk(Wv).astype(NPBF16),
        "Wo": _chunk(Wo).astype(NPBF16),
        "boe": (bv @ Wo + bo).reshape(1, C).astype(NPBF16),
        "lng": ln_g.reshape(1, C).astype(NPBF16),
        "lnb": ln_b.reshape(1, C).astype(NPBF16),
    }
    in_maps = []
    for core in range(8):
        b, half = core // 2, core % 2
        xt = np.ascontiguousarray(x[b].T)  # [C, T]
        pk = perms[b][:KL]
        mfp = mask[b][pk].astype(f32)     # permuted/truncated key mask
        vcol = np.zeros((P, NC), f32)
        vcol[:, :NK] = mfp.reshape(NK, P).T
        vecs = np.stack([bq.reshape(NC, P).T, bk.reshape(NC, P).T, vcol], axis=-1)
        m = dict(shared)
        m["xT"] = _chunk(np.ascontiguousarray(xt[:, pk])).astype(NPBF16)
        m["xTq"] = _chunk(xt[:, half * TQ : (half + 1) * TQ]).astype(NPBF16)
        m["vecs"] = np.ascontiguousarray(vecs, f32)
        m["maskv"] = np.ascontiguousarray(mfp.reshape(NK, P).T.astype(NPBF16))
        in_maps.append(m)

    trace = os.environ.get("KERNEL_TRACE", "0") == "1"
    if trace:
        _ensure_ntff_hook()
    LAST_RESULTS = run_bass_kernel_spmd(
        nc, in_maps, core_ids=list(range(8)), trace=trace
    )
    out = np.empty((B, T, C), f32)
    for core in range(8):
        b, half = core // 2, core % 2
        out[b, half * TQ : (half + 1) * TQ, :] = LAST_RESULTS.results[core][
            "y"
        ].astype(f32)
    return out


# revision 49
# speedup vs baseline: 1.0944x; 1.0029x over previous
"""Trainium2 Bass kernel for nn_MultiHeadAttention (B=4,T=1024,C=1024,H=16).

Sharding: 8 cores = 4 batches x 2 query-halves. Each core computes, for its
batch b and its 512 query rows:
  V projection (natural layout, mask folded in, + 64 replicated mask cols per
  head so the O matmul emits the softmax denominator pre-replicated across 64
  psum partitions), then per head-pair: Q^T/K^T projection chunks,
  S^T = K^T.T @ Q^T (row-packed head pairs, D=64 contraction), E^T =
  exp(0.125*S^T) on ACT (unmasked), O^T+den via one [V_h*m | m*64] matmul,
  normalize with reciprocal_approx_fast at partition base 0 (no DRAM bounce);
  finally Y = O^T.T @ Wo + (bv@Wo+bo) (bias via K=1 matmul) and LayerNorm with
  stats on PSUM, affine on ACT, bf16 output.
Host gathers the 8 [512,1024] bf16 outputs into [4,1024,1024] f32.
"""

import os
import sys

import numpy as np

for _p in ("/opt/trn_rl_repo", "/root/.axon_site/_ro/trn_rl_repo"):
    if os.path.isdir(_p) and _p not in sys.path:
        sys.path.append(_p)

import ml_dtypes  # noqa: E402
import concourse.bass as bass  # noqa: E402
import concourse.mybir as mybir  # noqa: E402
import concourse.tile as tile  # noqa: E402
from concourse import bacc  # noqa: E402
from concourse.bass_utils import run_bass_kernel_spmd  # noqa: E402

BF16 = mybir.dt.bfloat16
F32 = mybir.dt.float32
NPBF16 = ml_dtypes.bfloat16

B, T, C, H = 4, 1024, 1024, 16
D = C // H          # 64
P = 128             # partitions
NC = C // P         # 8 chunks of C
NT = T // P         # 8 chunks of T
TQ = T // 2         # 512 query rows per core
NQ = TQ // P        # 4 query chunks
NPAIR = H // 2      # 8 head pairs
EPS = 1e-5

_CACHE = {}
LAST_RESULTS = None


def _ensure_ntff_hook():
    """Register the axon NTFF profiling hook if the image's antenv lacks it."""
    try:
        import antenv.axon_hooks  # noqa: F401
        return
    except ImportError:
        pass
    try:
        import types

        import antenv
        from trn_agent_boot.trn_boot import _ntff_profile_via_ctypes

        mod = types.ModuleType("antenv.axon_hooks")
        state = {"hook": None}
        mod.set_axon_ntff_profile_hook = lambda h: state.__setitem__("hook", h)
        mod.get_axon_ntff_profile_hook = lambda: state["hook"]
        sys.modules["antenv.axon_hooks"] = mod
        antenv.axon_hooks = mod
        hook = _ntff_profile_via_ctypes("/opt/axon/libaxon_pjrt.so")
        if hook is not None:
            mod.set_axon_ntff_profile_hook(hook)
    except Exception:
        pass


def _emit(nc, tc, dr, NK, triv_gb, triv_boe):
    """Emit the per-core Tile program."""
    from contextlib import ExitStack

    with ExitStack() as ctx:
        consts = ctx.enter_context(tc.tile_pool(name="consts", bufs=1))

        KL = NK * P
        # VA[:, jc, h, :] = [V_h*m | m x64] for even h (O rows 0:64, den64
        # rows 64:128), [m x64 | V_h*m] for odd h (den64 low, O high). The
        # mask block gives the softmax denominator replicated across 64 psum
        # partitions for free in the O matmul.
        VA = consts.tile([P, NK, H, P], BF16)
        OT = consts.tile([P, NC, TQ], BF16)        # O^T (unnormed even/odd)
        Wo_sb = consts.tile([P, NC, C], BF16)
        lng_rep = lnb_rep = boe_sb = ones1 = None
        if not triv_gb:
            lng_rep = consts.tile([P, C], BF16)
            lnb_rep = consts.tile([P, C], BF16)
        if not triv_boe:
            boe_sb = consts.tile([1, C], BF16)     # bv@Wo + bo
            ones1 = consts.tile([1, P], BF16)
        vecs = consts.tile([P, NC, 3], F32)        # bq | bk | maskf
        maskv = consts.tile([P, NK], BF16)
        eps_t = consts.tile([P, 1], F32)

        nc.vector.memset(eps_t, EPS)
        if ones1 is not None:
            nc.vector.memset(ones1, 1.0)

        with (
            tc.tile_pool(name="pa", bufs=1) as pa,
            tc.tile_pool(name="pb", bufs=2) as pb,
            tc.tile_pool(name="psP", bufs=2, space="PSUM") as psP,
        ):
            xT = pa.tile([P, NC, KL], BF16)
            xTq = pa.tile([P, NC, TQ], BF16)
            Wq_sb = pa.tile([P, NC, C], BF16)
            Wk_sb = pa.tile([P, NC, C], BF16)
            Wv_sb = pa.tile([P, NC, C], BF16)
            # Input loads striped across the three DMA queues (sync/scalar/
            # gpsimd each stripe over all 16 DMA engines; aggregate ~350GB/s),
            # phased by first-use: xT+Wv lo | Wv hi | xTq+Wq | Wk | Wo.
            ENG = [nc.sync, nc.scalar, nc.gpsimd]

            def _ldsplit(dst, src):
                n = dst.shape[1]
                bnd = [0, (n + 2) // 3, n - (n + 2) // 3, n]
                bnd[2] += bnd[1]
                for e in range(3):
                    lo, hi = bnd[e], bnd[e + 1]
                    if hi > lo:
                        ENG[e].dma_start(
                            out=dst[:, lo:hi], in_=src[:, lo:hi]
                        )

            # V-proj inputs interleaved per kc chunk so mm(kc) can start as
            # soon as its own xT row + Wv column block land (~10.5us).
            for kc in range(NC):
                ENG[kc % 3].dma_start(
                    out=xT[:, kc, :], in_=dr["xT"].ap()[:, kc, :]
                )
                ENG[kc % 3].dma_start(
                    out=Wv_sb[:, kc, 0:TQ], in_=dr["Wv"].ap()[:, kc, 0:TQ]
                )
            nc.gpsimd.dma_start(out=vecs[:], in_=dr["vecs"].ap()[:])
            nc.gpsimd.dma_start(out=maskv[:], in_=dr["maskv"].ap()[:])
            if not triv_boe:
                nc.gpsimd.dma_start(out=boe_sb[:], in_=dr["boe"].ap()[:])
            _ldsplit(Wv_sb[:, :, TQ:], dr["Wv"].ap()[:, :, TQ:])
            _ldsplit(Wk_sb[:], dr["Wk"].ap()[:])
            _ldsplit(xTq[:], dr["xTq"].ap()[:])
            _ldsplit(Wq_sb[:], dr["Wq"].ap()[:])
            _ldsplit(Wo_sb[:], dr["Wo"].ap()[:])
            if not triv_gb:
                for name, rep in (("lng", lng_rep), ("lnb", lnb_rep)):
                    a = dr[name].ap()
                    bcast = bass.AP(
                        tensor=a.tensor, offset=a.offset, ap=[[0, P], [1, C]]
                    )
                    nc.gpsimd.dma_start(out=rep[:], in_=bcast)

            # ---- output projection + LayerNorm body (called per qc) ----
            def _qc_head(qc, psn):
                # mc 0..6 accumulation for both halves (no reads of the last
                # pair's OT chunk) — bridge work for the tensor queue.
                qs = slice(qc * P, (qc + 1) * P)
                for nn in range(2):
                    for mc in range(NC - 1):
                        nc.tensor.matmul(
                            psn[nn],
                            OT[:, mc, qs],
                            Wo_sb[:, mc, nn * TQ : (nn + 1) * TQ],
                            start=(mc == 0),
                            stop=False,
                        )

            def _qc_tail(qc, psn):
                qs = slice(qc * P, (qc + 1) * P)
                for nn in range(2):
                    nc.tensor.matmul(
                        psn[nn],
                        OT[:, NC - 1, qs],
                        Wo_sb[:, NC - 1, nn * TQ : (nn + 1) * TQ],
                        start=False, stop=triv_boe,
                    )
                    if not triv_boe:
                        nc.tensor.matmul(
                            psn[nn],
                            ones1[0:1, :],
                            boe_sb[0:1, nn * TQ : (nn + 1) * TQ],
                            start=False, stop=True,
                        )
                stats = pb.tile(
                    [P, 2, nc.vector.BN_STATS_DIM], F32, tag="stats"
                )
                mv = pb.tile([P, nc.vector.BN_AGGR_DIM], F32, tag="mv")
                nc.vector.bn_stats(out=stats[:, 0, :], in_=psn[0][:])
                nc.vector.bn_stats(out=stats[:, 1, :], in_=psn[1][:])
                nc.vector.bn_aggr(out=mv[:], in_=stats[:])
                rstd = pb.tile([P, 1], F32, tag="rstd")
                nmr = pb.tile([P, 1], F32, tag="nmr")
                nc.scalar.activation(
                    out=rstd[:], in_=mv[:, 1:2],
                    func=mybir.ActivationFunctionType.Sqrt,
                    bias=eps_t[:], scale=1.0,
                )
                nc.vector.reciprocal(out=rstd[:], in_=rstd[:])
                nc.vector.tensor_scalar(
                    nmr[:], mv[:, 0:1], rstd[:], -1.0,
                    mybir.AluOpType.mult, mybir.AluOpType.mult,
                )
                Yf = pb.tile([P, C], BF16, tag="yf")
                for nn in range(2):
                    sl = slice(nn * TQ, (nn + 1) * TQ)
                    # split the affine across engines so the two halves of
                    # each qc run concurrently in the tail
                    if nn == 0:
                        nc.scalar.activation(
                            out=Yf[:, sl], in_=psn[nn][:],
                            func=mybir.ActivationFunctionType.Identity,
                            bias=nmr[:], scale=rstd[:],
                        )
                    else:
                        nc.vector.tensor_scalar(
                            Yf[:, sl], psn[nn][:], rstd[:], nmr[:],
                            mybir.AluOpType.mult, mybir.AluOpType.add,
                        )
                    if not triv_gb:
                        if not triv_gb:
                            nc.vector.tensor_tensor(
                                Yf[:, sl], Yf[:, sl], lng_rep[:, sl],
                                mybir.AluOpType.mult,
                            )
                            nc.vector.tensor_tensor(
                                Yf[:, sl], Yf[:, sl], lnb_rep[:, sl],
                                mybir.AluOpType.add,
                            )
                    (nc.sync if nn == 0 else nc.gpsimd).dma_start(
                        out=dr["y"].ap()[qs, sl], in_=Yf[:, sl]
                    )

            with (
                tc.tile_pool(name="psS", bufs=2, space="PSUM") as psS,
                tc.tile_pool(name="psO", bufs=2, space="PSUM") as psO,
            ):
                # ---- V projection: [keys, C] natural, masked, + mask block --
                for nn in range(2):
                    for tcn in range(NK):
                        ps = psP.tile([P, TQ], F32, tag="psp")
                        for kc in range(NC):
                            nc.tensor.matmul(
                                ps[:],
                                xT[:, kc, tcn * P : (tcn + 1) * P],
                                Wv_sb[:, kc, nn * TQ : (nn + 1) * TQ],
                                start=(kc == 0),
                                stop=(kc == NC - 1),
                            )
                        # V blocks land at +0 (even heads) / +64 (odd heads)
                        a = VA[:, tcn, :, :]
                        vdst = bass.AP(
                            tensor=a.tensor, offset=a.offset + nn * 8 * P,
                            ap=[a.ap[0], [2 * P, 4], [P + D, 2], [1, D]],
                        )
                        nc.vector.tensor_scalar_mul(
                            vdst,
                            ps[:].rearrange("p (a b d) -> p a b d", a=4, b=2),
                            vecs[:, tcn, 2:3],
                        )
                        if nn == 0:
                            # mask blocks: +64 for even heads, +0 for odd
                            mdst = bass.AP(
                                tensor=a.tensor, offset=a.offset + D,
                                ap=[a.ap[0], [2 * P, 8], [D, 2], [1, D]],
                            )
                            nc.vector.tensor_copy(
                                out=mdst,
                                in_=maskv[:, tcn, None].to_broadcast(
                                    (P, 8, 2, D)
                                ),
                            )

                # ---- per head-pair: QT/KT proj, S^T, exp, O^T, normalize,
                # software-pipelined so K/Q(c+1) matmuls cover exp(c) on ACT.
                def emit_kq(c):
                    QTc = pb.tile([P, TQ], BF16, tag="qtc")
                    KTc = pb.tile([P, KL], BF16, tag="ktc")
                    for ko in range(0, KL, TQ):
                        w = min(TQ, KL - ko)
                        ps = psP.tile([P, TQ], F32, tag="psp")
                        for kc in range(NC):
                            nc.tensor.matmul(
                                ps[:, :w],
                                Wk_sb[:, kc, c * P : (c + 1) * P],
                                xT[:, kc, ko : ko + w],
                                start=(kc == 0),
                                stop=(kc == NC - 1),
                            )
                        nc.vector.tensor_scalar_add(
                            KTc[:, ko : ko + w], ps[:, :w], vecs[:, c, 1:2]
                        )
                    ps = psP.tile([P, TQ], F32, tag="psp")
                    for kc in range(NC):
                        nc.tensor.matmul(
                            ps[:],
                            Wq_sb[:, kc, c * P : (c + 1) * P],
                            xTq[:, kc, :],
                            start=(kc == 0),
                            stop=(kc == NC - 1),
                        )
                    nc.vector.tensor_scalar_add(QTc[:], ps[:], vecs[:, c, 0:1])
                    return QTc, KTc

                def emit_s(c, QTc, KTc):
                    et0 = pb.tile([P, NK, TQ], BF16, tag="et0", bufs=2)
                    et1 = pb.tile([P, NK, TQ], BF16, tag="et1", bufs=2)
                    for jj in range((NK + 1) // 2):
                        nu = min(2, NK - 2 * jj)
                        s0 = psS.tile([P, 2 * TQ], F32, tag="s0", bufs=1)
                        s1 = psS.tile([P, 2 * TQ], F32, tag="s1", bufs=1)
                        for u in range(nu):
                            jc = 2 * jj + u
                            js = slice(jc * P, (jc + 1) * P)
                            nc.tensor.matmul(
                                s0[:, u * TQ : (u + 1) * TQ],
                                KTc[0:D, js],
                                QTc[0:D, :],
                                start=True, stop=True,
                                tile_position=(0, 0),
                            )
                            nc.tensor.matmul(
                                s1[:, u * TQ : (u + 1) * TQ],
                                KTc[D:P, js],
                                QTc[D:P, :],
                                start=True, stop=True,
                                tile_position=(D, 0),
                            )
                        nc.scalar.activation(
                            out=et0[:, 2 * jj : 2 * jj + nu, :],
                            in_=s0[:, : nu * TQ],
                            func=mybir.ActivationFunctionType.Exp, scale=0.125,
                        )
                        nc.scalar.activation(
                            out=et1[:, 2 * jj : 2 * jj + nu, :],
                            in_=s1[:, : nu * TQ],
                            func=mybir.ActivationFunctionType.Exp, scale=0.125,
                        )
                    return et0, et1

                def emit_o_den(c, et0, et1):
                    h0, h1 = 2 * c, 2 * c + 1
                    # O^T + den64: even head [O | den64], odd [den64 | O].
                    # The even head's den chain (ACT copy -> DMA shift ->
                    # recip) is emitted between the two O matmul groups so it
                    # overlaps the odd head's matmuls.
                    po0 = psO.tile([P, TQ], F32, tag="po0", bufs=1)
                    po1 = psO.tile([P, TQ], F32, tag="po1", bufs=1)
                    dt = pb.tile([P, 2, TQ], F32, tag="dt")
                    rr = pb.tile([P, 2, TQ], F32, tag="rr")
                    for jc in range(NK):
                        nc.tensor.matmul(
                            po0[:], VA[:, jc, h0, :], et0[:, jc, :],
                            start=(jc == 0), stop=(jc == NK - 1),
                        )
                    nc.scalar.activation(
                        out=dt[D:P, 0, :], in_=po0[D:P, :],
                        func=mybir.ActivationFunctionType.Identity,
                    )
                    nc.sync.dma_start(out=dt[0:D, 0, :], in_=dt[D:P, 0, :])
                    nc.vector.reciprocal_approx_fast(
                        out=rr[0:D, 0, :], in_=dt[0:D, 0, :]
                    )
                    for jc in range(NK):
                        nc.tensor.matmul(
                            po1[:], VA[:, jc, h1, :], et1[:, jc, :],
                            start=(jc == 0), stop=(jc == NK - 1),
                        )
                    nc.vector.reciprocal_approx_fast(
                        out=rr[0:D, 1, :], in_=po1[0:D, :]
                    )
                    nc.sync.dma_start(out=rr[D:P, 1, :], in_=rr[0:D, 1, :])
                    nc.vector.tensor_tensor(
                        OT[0:D, c, :], po0[0:D, :], rr[0:D, 0, :],
                        mybir.AluOpType.mult,
                    )
                    nc.vector.tensor_tensor(
                        OT[D:P, c, :], po1[D:P, :], rr[D:P, 1, :],
                        mybir.AluOpType.mult,
                    )

                ets = emit_s(0, *emit_kq(0))
                for c in range(NPAIR):
                    nxt = emit_kq(c + 1) if c + 1 < NPAIR else None
                    emit_o_den(c, *ets)
                    if nxt is not None:
                        ets = emit_s(c + 1, *nxt)

                # qc 0/1 emitted inside the psS/psO scope from psP so their
                # matmuls bridge the pool-close barrier (which waits on the
                # last pair's den chain); the rest pipeline deeply from psL.
                # Bridge the pool-close barrier: four mc0..6 chains emitted
                # before any mc7 tail (the in-order tensor queue would
                # otherwise block on the last pair's OT chunk). qc1's psums
                # reuse the psS slots, free right after pair 7's exp.
                a0 = psP.tile([P, TQ], F32, tag="psp")
                a1 = psP.tile([P, TQ], F32, tag="psp")
                b0 = psO.tile([P, TQ], F32, tag="po0", bufs=1)
                b1 = psO.tile([P, TQ], F32, tag="po1", bufs=1)
                qc0 = [a0[:], a1[:]]
                qc1 = [b0[:], b1[:]]
                _qc_head(0, qc0)
                _qc_head(1, qc1)
                _qc_tail(0, qc0)
                _qc_tail(1, qc1)
            with tc.tile_pool(name="psL", bufs=4, space="PSUM") as psL:
                for qc in range(2, NQ):
                    p0 = psL.tile([P, TQ], F32, tag="psl")
                    p1 = psL.tile([P, TQ], F32, tag="psl")
                    psn = [p0[:], p1[:]]
                    if qc < NQ - 1:
                        _qc_head(qc, psn)
                        _qc_tail(qc, psn)
                        continue
                    # last qc: finish half 0 completely first so its
                    # bn_stats overlaps half 1's matmuls, shortening the
                    # exposed end-of-kernel LN chain.
                    qs = slice(qc * P, (qc + 1) * P)
                    stats = pb.tile(
                        [P, 2, nc.vector.BN_STATS_DIM], F32, tag="stats"
                    )
                    for nn in range(2):
                        for mc in range(NC):
                            nc.tensor.matmul(
                                psn[nn],
                                OT[:, mc, qs],
                                Wo_sb[:, mc, nn * TQ : (nn + 1) * TQ],
                                start=(mc == 0),
                                stop=(triv_boe and mc == NC - 1),
                            )
                        if not triv_boe:
                            nc.tensor.matmul(
                                psn[nn],
                                ones1[0:1, :],
                                boe_sb[0:1, nn * TQ : (nn + 1) * TQ],
                                start=False, stop=True,
                            )
                        nc.vector.bn_stats(out=stats[:, nn, :], in_=psn[nn])
                    mv = pb.tile([P, nc.vector.BN_AGGR_DIM], F32, tag="mv")
                    nc.vector.bn_aggr(out=mv[:], in_=stats[:])
                    rstd = pb.tile([P, 1], F32, tag="rstd")
                    nmr = pb.tile([P, 1], F32, tag="nmr")
                    nc.scalar.activation(
                        out=rstd[:], in_=mv[:, 1:2],
                        func=mybir.ActivationFunctionType.Sqrt,
                        bias=eps_t[:], scale=1.0,
                    )
                    nc.vector.reciprocal(out=rstd[:], in_=rstd[:])
                    nc.vector.tensor_scalar(
                        nmr[:], mv[:, 0:1], rstd[:], -1.0,
                        mybir.AluOpType.mult, mybir.AluOpType.mult,
                    )
                    Yf = pb.tile([P, C], BF16, tag="yf")
                    for nn in range(2):
                        sl = slice(nn * TQ, (nn + 1) * TQ)
                        if nn == 0:
                            nc.scalar.activation(
                                out=Yf[:, sl], in_=psn[nn],
                                func=mybir.ActivationFunctionType.Identity,
                                bias=nmr[:], scale=rstd[:],
                            )
                        else:
                            nc.vector.tensor_scalar(
                                Yf[:, sl], psn[nn], rstd[:], nmr[:],
                                mybir.AluOpType.mult, mybir.AluOpType.add,
                            )
                        if not triv_gb:
                            nc.vector.tensor_tensor(
                                Yf[:, sl], Yf[:, sl], lng_rep[:, sl],
                                mybir.AluOpType.mult,
                            )
                            nc.vector.tensor_tensor(
                                Yf[:, sl], Yf[:, sl], lnb_rep[:, sl],
                                mybir.AluOpType.add,
                            )
                        (nc.sync if nn == 0 else nc.gpsimd).dma_start(
                            out=dr["y"].ap()[qs, sl], in_=Yf[:, sl]
                        )


def _build(NK=NT, triv_gb=False, triv_boe=False):
    nc = bacc.Bacc("TRN2", target_bir_lowering=False, debug=False, num_devices=8)
    dr = {}
    dr["xT"] = nc.dram_tensor("xT", [P, NC, NK * P], BF16, kind="ExternalInput")
    dr["xTq"] = nc.dram_tensor("xTq", [P, NC, TQ], BF16, kind="ExternalInput")
    for w in ("Wq", "Wk", "Wv", "Wo"):
        dr[w] = nc.dram_tensor(w, [P, NC, C], BF16, kind="ExternalInput")
    dr["vecs"] = nc.dram_tensor("vecs", [P, NC, 3], F32, kind="ExternalInput")
    dr["maskv"] = nc.dram_tensor("maskv", [P, NK], BF16, kind="ExternalInput")
    dr["boe"] = nc.dram_tensor("boe", [1, C], BF16, kind="ExternalInput")
    for v in ("lng", "lnb"):
        dr[v] = nc.dram_tensor(v, [1, C], BF16, kind="ExternalInput")
    dr["y"] = nc.dram_tensor("y", [TQ, C], BF16, kind="ExternalOutput")
    with tile.TileContext(nc) as tc:
        _emit(nc, tc, dr, NK, triv_gb, triv_boe)
    nc.compile()
    return nc


def _chunk(a):
    """[C, N] -> [128, C//128, N] with [p, c, n] = a[128c+p, n]."""
    return np.ascontiguousarray(
        a.reshape(NC, P, -1).transpose(1, 0, 2)
    )


def kernel(**inputs):
    global LAST_RESULTS
    f32 = np.float32
    Wq = np.asarray(inputs["Wq"], f32)
    Wk = np.asarray(inputs["Wk"], f32)
    Wv = np.asarray(inputs["Wv"], f32)
    Wo = np.asarray(inputs["Wo"], f32)
    x = np.asarray(inputs["x"], f32)
    mask = np.asarray(inputs["attn_mask"]).reshape(B, T)
    # sort keys so unmasked come first; masked tail chunks are dropped
    perms = [np.argsort(-mask[b], kind="stable") for b in range(B)]
    m1max = max(int(mask[b].sum()) for b in range(B))
    NK = min(NT, max(1, -(-m1max // P)))
    KL = NK * P
    bq = np.asarray(inputs["bq"], f32)
    bk = np.asarray(inputs["bk"], f32)
    bv = np.asarray(inputs["bv"], f32)
    bo = np.asarray(inputs["bo"], f32)
    ln_g = np.asarray(inputs["ln_g"], f32)
    ln_b = np.asarray(inputs["ln_b"], f32)
    boe_v = (bv @ Wo + bo).astype(f32)
    triv_gb = bool(np.all(ln_g == 1.0) and np.all(ln_b == 0.0))
    triv_boe = bool(np.all(boe_v == 0.0))
    key = ("nc", NK, triv_gb, triv_boe)
    if key not in _CACHE:
        _CACHE[key] = _build(NK=NK, triv_gb=triv_gb, triv_boe=triv_boe)
    nc = _CACHE[key]

    shared = {
        "Wq": _chunk(Wq).astype(NPBF16),
        "Wk": _chunk(Wk).astype(NPBF16),
        "Wv": _chunk(Wv).astype(NPBF16),
        "Wo": _chunk(Wo).astype(NPBF16),
        "boe": (bv @ Wo + bo).reshape(1, C).astype(NPBF16),
        "lng": ln_g.reshape(1, C).astype(NPBF16),
        "lnb": ln_b.reshape(1, C).astype(NPBF16),
    }
    in_maps = []
    for core in range(8):
        b, half = core // 2, core % 2
        xt = np.ascontiguousarray(x[b].T)  # [C, T]
        pk = perms[b][:KL]
        mfp = mask[b][pk].astype(f32)     # permuted/truncated key mask
        vcol = np.zeros((P, NC), f32)
        vcol[:, :NK] = mfp.reshape(NK, P).T
        vecs = np.stack([bq.reshape(NC, P).T, bk.reshape(NC, P).T, vcol], axis=-1)
        m = dict(shared)
        m["xT"] = _chunk(np.ascontiguousarray(xt[:, pk])).astype(NPBF16)
        m["xTq"] = _chunk(xt[:, half * TQ : (half + 1) * TQ]).astype(NPBF16)
        m["vecs"] = np.ascontiguousarray(vecs, f32)
        m["maskv"] = np.ascontiguousarray(mfp.reshape(NK, P).T.astype(NPBF16))
        in_maps.append(m)

    trace = os.environ.get("KERNEL_TRACE", "0") == "1"
    if trace:
        _ensure_ntff_hook()
    LAST_RESULTS = run_bass_kernel_spmd(
        nc, in_maps, core_ids=list(range(8)), trace=trace
    )
    out = np.empty((B, T, C), f32)
    for core in range(8):
        b, half = core // 2, core % 2
        out[b, half * TQ : (half + 1) * TQ, :] = LAST_RESULTS.results[core][
            "y"
        ].astype(f32)
    return out
